# revision 3
# baseline (speedup 1.0000x reference)
"""Trainium2 Bass kernel for nn_BasicDeconvolutionBlock.

Reference computation (see problem statement):
    gathered = feats[in_map]                         # [K, M, Cin]
    contrib  = einsum('kmc,kcd->kmd', gathered, W)   # [K, M, Cout]
    out      = zeros([n_out, Cout]).at[out_map].add(contrib)
    y        = relu(batchnorm(out))                  # batch stats over n_out rows

Strategy (8 NeuronCores, SPMD):
  - Host routes each (k, m) pair to the core owning its output row
    (row blocks of n_out/8).  Per core ~169k pairs.
  - Gather: feats pre-cast to bf16, padded to 128 channels (256B rows).
    dma_gather(transpose=True) produces a CHANNEL-MAJOR SBUF slab
    G[128ch, slots] directly.  int16 gather indices -> feats is split in
    chunks of 32768 rows; pairs are grouped by (chunk, k), groups padded
    to a multiple of 128 slots.
  - GEMM: per 128-slot tile, matmul(lhsT=G_tile[128ch,128slots] (stationary),
    rhs=Wpad[k][128ch,64]) -> PSUM contrib[128slots, 64] fp32 (m-major,
    no transposes anywhere).
  - Scatter: DVE copies PSUM->SBUF slab, then gpsimd dma_scatter_add
    (CCE-add, int16 idx) accumulates rows into one of two HBM accumulator
    banks (cycled by round parity so chains overlap).  Duplicate rows race
    in hardware, so a host-side occurrence-round split guarantees unique
    rows per call; same-bank calls serialize via Tile WAW deps.  SWDGE
    calls are capped at 896 indices (the Q7 ucode descriptor-ring limit;
    larger calls hard-wedge the device).
  - BN: ones-matmul row sums + sum of squares, [2,64] AllReduce across
    the 8 cores, normalize + ReLU on chip, output shard [rows,64] fp32.
"""

import os
import sys

import numpy as np

sys.path.insert(0, "/opt/trn_rl_repo")

import ml_dtypes  # noqa: E402

from concourse import bacc, bass, mybir  # noqa: E402
import concourse.tile as tile  # noqa: E402

BN_EPS = 1e-5
CHUNK = 32768  # int16 gather index range per feats chunk
SEG_SLOTS = int(os.environ.get("DECONV_SEG_SLOTS", "1920"))
DMA_SCRATCH = int(os.environ.get("DECONV_DMA_SCRATCH", "65536"))
F32 = mybir.dt.float32
BF16 = mybir.dt.bfloat16
I16 = mybir.dt.int16
I32 = mybir.dt.int32


def _roundup(x, m):
    return (x + m - 1) // m * m


def _route(in_map, out_map, n_out, n_cores, dup_safe, expand=1):
    """Host-side routing. Returns compile-time plan + per-core packed arrays.

    Slot stream per core: for r in rounds, for c in chunks, for k in K:
    group (r,c,k) padded to a multiple of 128 slots.  If dup_safe, a single
    round (r=0) is used (occurrence splitting disabled).

    expand=E spreads a row's duplicate contributions over E contiguous
    accumulator banks (phys row = (occ%E)*acc_rows + row, round = occ//E),
    halving/quartering the round count; the kernel folds banks before BN.
    """
    K, M = in_map.shape
    rows_per_core = n_out // n_cores
    assert rows_per_core * n_cores == n_out
    acc_rows = _roundup(rows_per_core, 128)
    nchunk = _roundup(int(in_map.max()) + 1, CHUNK) // CHUNK

    k_idx = np.repeat(np.arange(K, dtype=np.int32), M)
    in_flat = in_map.ravel().astype(np.int64)
    out_flat = out_map.ravel().astype(np.int64)
    core = out_flat // rows_per_core
    row_local = (out_flat - core * rows_per_core).astype(np.int32)
    chunk = (in_flat // CHUNK).astype(np.int32)
    idx_local = (in_flat - chunk.astype(np.int64) * CHUNK).astype(np.int32)

    per_core = []
    max_round = 1
    for c in range(n_cores):
        sel = np.nonzero(core == c)[0]
        rows_c = row_local[sel]
        if dup_safe:
            rnd = np.zeros(len(sel), dtype=np.int32)
            prow = rows_c.astype(np.int32)
        else:
            order = np.argsort(rows_c, kind="stable")
            sr = rows_c[order]
            n = len(sr)
            first = np.ones(n, dtype=bool)
            first[1:] = sr[1:] != sr[:-1]
            grp_start = np.maximum.accumulate(np.where(first, np.arange(n), 0))
            occ_sorted = np.arange(n) - grp_start
            occ = np.empty(n, dtype=np.int64)
            occ[order] = occ_sorted
            rnd = (occ // expand).astype(np.int32)
            prow = (rows_c + (occ % expand) * acc_rows).astype(np.int32)
            max_round = max(max_round, int(rnd.max()) + 1 if n else 1)
        per_core.append(
            dict(rnd=rnd, chunk=chunk[sel], k=k_idx[sel],
                 idx=idx_local[sel], row=prow)
        )

    R = max_round
    # group counts [R, nchunk, K] per core -> shared caps
    counts = np.zeros((n_cores, R, nchunk, K), dtype=np.int64)
    for c in range(n_cores):
        p = per_core[c]
        np.add.at(counts[c], (p["rnd"], p["chunk"], p["k"]), 1)
    caps = (np.ceil(counts.max(axis=0) / 128).astype(np.int64) * 128)  # [R,nchunk,K]

    # segments: contiguous runs of (r,c,k) group pieces, same (r,c),
    # <= SEG_SLOTS per segment (SWDGE per-instruction descriptor limit).
    # Groups larger than SEG_SLOTS are split across segments.
    segments = []  # dicts: r, c, slot0 (global), nslots, groups=[(k, len, off_in_seg)]
    group_slot0 = {}  # (r,c,k) -> global slot of the group's first slot
    slot0 = 0
    for r in range(R):
        for c in range(nchunk):
            cur = None
            for k in range(K):
                cap = int(caps[r, c, k])
                if cap == 0:
                    continue
                group_slot0[(r, c, k)] = slot0 + (cur["nslots"] if cur else 0)
                rem = cap
                while rem > 0:
                    if cur is None:
                        cur = dict(r=r, c=c, slot0=slot0, nslots=0, groups=[])
                    take = min(SEG_SLOTS - cur["nslots"], rem)
                    if take == 0:
                        segments.append(cur)
                        slot0 += cur["nslots"]
                        cur = None
                        continue
                    cur["groups"].append((k, take, cur["nslots"]))
                    cur["nslots"] += take
                    rem -= take
            if cur is not None:
                segments.append(cur)
                slot0 += cur["nslots"]
                cur = None
    total_slots = slot0

    dump_row = expand * acc_rows  # rows beyond the banks are the dump zone
    acc_total = expand * acc_rows + 128

    # pack per-core gather idx and scatter idx (both int16, wrapped 16)
    gcols = sum(seg["nslots"] // 16 for seg in segments)
    scols = gcols
    gidx_all = np.zeros((n_cores, 128, gcols), dtype=np.int16)
    sidx_all = np.full((n_cores, 128, scols), dump_row, dtype=np.int16)

    seg_gcol0 = []
    seg_scol0 = []
    g0 = s0 = 0
    for seg in segments:
        seg_gcol0.append(g0)
        seg_scol0.append(s0)
        g0 += seg["nslots"] // 16
        s0 += seg["nslots"] // 16

    for cidx in range(n_cores):
        p = per_core[cidx]
        order = np.lexsort((p["row"], p["k"], p["chunk"], p["rnd"]))
        rnd_s, ch_s, k_s = p["rnd"][order], p["chunk"][order], p["k"][order]
        idx_s, row_s = p["idx"][order], p["row"][order]
        # slot of each pair: group_slot0 + position within group
        key = (rnd_s.astype(np.int64) * nchunk + ch_s) * K + k_s
        n = len(key)
        first = np.ones(n, dtype=bool)
        first[1:] = key[1:] != key[:-1]
        grp_start = np.maximum.accumulate(np.where(first, np.arange(n), 0))
        pos_in_grp = np.arange(n) - grp_start
        base = np.array(
            [group_slot0[(int(r_), int(c_), int(k_))]
             for r_, c_, k_ in zip(rnd_s[first], ch_s[first], k_s[first])],
            dtype=np.int64,
        )
        base_full = np.repeat(base, np.diff(np.nonzero(
            np.concatenate([first, [True]]))[0]))
        slots = base_full + pos_in_grp

        gvals = np.zeros(total_slots, dtype=np.int16)
        svals = np.full(total_slots, dump_row, dtype=np.int16)
        gvals[slots] = idx_s.astype(np.int16)
        svals[slots] = row_s
        # per-segment packing
        for si, seg in enumerate(segments):
            a, b = seg["slot0"], seg["slot0"] + seg["nslots"]
            gseg = gvals[a:b].reshape(-1, 16).T  # [16, n/16]
            gidx_all[cidx, :, seg_gcol0[si]:seg_gcol0[si] + (b - a) // 16] = (
                np.tile(gseg, (8, 1)))
            sseg = np.tile(svals[a:b].astype(np.int16).reshape(-1, 16).T,
                           (8, 1))  # wrapped like gather idxs
            sidx_all[cidx, :, seg_scol0[si]:seg_scol0[si] + (b - a) // 16] = sseg

    plan = dict(
        R=R, nchunk=nchunk, K=K, rows_per_core=rows_per_core,
        acc_rows=acc_rows, acc_total=acc_total, dump_row=dump_row,
        expand=expand,
        segments=segments, seg_gcol0=seg_gcol0, seg_scol0=seg_scol0,
        gcols=gcols, scols=scols, total_slots=total_slots,
    )
    return plan, gidx_all, sidx_all


def _build(plan, n_out, ftab_rows, n_cores):
    """Trace the Bass program. Returns nc."""
    nc = bacc.Bacc("TRN2", target_bir_lowering=False, debug=False,
                   dynamic_dma_scratch_size=DMA_SCRATCH)

    R, nchunk, K = plan["R"], plan["nchunk"], plan["K"]
    acc_rows, acc_total = plan["acc_rows"], plan["acc_total"]
    segments = plan["segments"]
    Cout = 64

    ftab = nc.dram_tensor("ftab", [ftab_rows, 128], BF16, kind="ExternalInput")
    wt = nc.dram_tensor("wt", [128, K * Cout], BF16, kind="ExternalInput")
    gidx = nc.dram_tensor("gidx", [128, plan["gcols"]], I16, kind="ExternalInput")
    sidx = nc.dram_tensor("sidx", [128, plan["scols"]], I16, kind="ExternalInput")
    gb = nc.dram_tensor("gb", [2, Cout], F32, kind="ExternalInput")
    # two accumulator banks cycled by round parity: scatter calls to
    # different banks have no WAW conflict, so adjacent rounds overlap
    acc0 = nc.dram_tensor("acc0", [acc_total, Cout], F32)
    acc1 = nc.dram_tensor("acc1", [acc_total, Cout], F32)
    accs = [acc0, acc1]
    cc_in = nc.dram_tensor("cc_in", [2, Cout], F32)
    cc_out = nc.dram_tensor("cc_out", [2, Cout], F32, addr_space="Shared")
    y = nc.dram_tensor("y", [acc_rows, Cout], F32, kind="ExternalOutput")

    Tb = acc_rows // 128  # BN column tiles

    with tile.TileContext(nc) as tc:
        with (
            tc.tile_pool(name="const", bufs=1) as cpool,
            tc.tile_pool(name="gpool", bufs=3) as gpool,
            tc.tile_pool(name="slab", bufs=3) as slabpool,
            tc.tile_pool(name="gixp", bufs=3) as gixpool,
            tc.tile_pool(name="sixp", bufs=3) as sixpool,
            tc.tile_pool(name="psum", bufs=8, space="PSUM") as pspool,
        ):
            # constants
            w_sb = cpool.tile([128, K * Cout], BF16, tag="w")
            nc.sync.dma_start(out=w_sb[:, :], in_=wt[:, :])
            zed = cpool.tile([128, 3200], F32, tag="zed")
            nc.vector.memset(zed[:, :], 0.0)
            # zero-init acc (acc_total*64 elems, in chunks of 128*3200)
            zrows = 128 * 3200 // Cout  # 6400 rows per DMA
            for bank in accs:
                r0 = 0
                while r0 < acc_total:
                    rcnt = min(zrows, acc_total - r0)
                    nc.sync.dma_start(
                        out=bank[r0:r0 + rcnt, :],
                        in_=zed[:, :rcnt * Cout // 128],
                    )
                    r0 += rcnt

            # main pipeline over segments
            for si, seg in enumerate(segments):
                ns = seg["nslots"]
                c = seg["c"]
                gi = gixpool.tile([128, SEG_SLOTS // 16], I16, tag="gi")
                nc.sync.dma_start(
                    out=gi[:, :ns // 16],
                    in_=gidx[:, plan["seg_gcol0"][si]:plan["seg_gcol0"][si] + ns // 16],
                )
                g = gpool.tile([128, 1, SEG_SLOTS], BF16, tag="g")
                nc.gpsimd.dma_gather(
                    out_ap=g[:, :, :ns],
                    in_ap=ftab[c * CHUNK:min((c + 1) * CHUNK, ftab_rows), :],
                    idxs_ap=gi[:, :ns // 16],
                    num_idxs=ns,
                    num_idxs_reg=ns,
                    elem_size=128,
                    transpose=True,
                )
                slab = slabpool.tile([128, SEG_SLOTS // 128, Cout], F32, tag="slab")
                for (k, cap, off) in seg["groups"]:
                    for j in range(cap // 128):
                        col = off + j * 128
                        ps = pspool.tile([128, Cout], F32, tag="ps")
                        nc.tensor.matmul(
                            out=ps[:, :],
                            lhsT=g[:, 0, col:col + 128],
                            rhs=w_sb[:, k * Cout:(k + 1) * Cout],
                            start=True, stop=True,
                        )
                        nc.vector.tensor_copy(
                            out=slab[:, col // 128, :], in_=ps[:, :])
                si_t = sixpool.tile([128, SEG_SLOTS // 16], I16, tag="si")
                nc.sync.dma_start(
                    out=si_t[:, :ns // 16],
                    in_=sidx[:, plan["seg_scol0"][si]:plan["seg_scol0"][si] + ns // 16],
                )
                nc.gpsimd.dma_scatter_add(
                    out_ap=accs[seg["r"] % 2][:, :],
                    in_ap=slab[:, :ns // 128, :],
                    idxs_ap=si_t[:, :ns // 16],
                    num_idxs=ns,
                    num_idxs_reg=ns,
                    elem_size=64,
                )

        # ---- BN phase ----
        with (
            tc.tile_pool(name="bn", bufs=1) as bnpool,
            tc.tile_pool(name="bns", bufs=4) as bnspool,
            tc.tile_pool(name="bnp", bufs=2, space="PSUM") as bnps,
        ):
            out_sb = bnpool.tile([128, Tb, 64], F32, tag="outsb")
            nc.sync.dma_start(out=out_sb[:, :, :], in_=acc0[0:acc_rows, :])
            bank_sb = bnpool.tile([128, Tb, 64], F32, tag="bank")
            nc.sync.dma_start(out=bank_sb[:, :, :], in_=acc1[0:acc_rows, :])
            nc.vector.tensor_tensor(
                out=out_sb[:, :, :], in0=out_sb[:, :, :],
                in1=bank_sb[:, :, :], op=mybir.AluOpType.add)
            ones = bnpool.tile([128, 1], F32, tag="ones")
            nc.vector.memset(ones[:, :], 1.0)
            sum_ps = bnps.tile([1, 64], F32, tag="sum")
            sq_ps = bnps.tile([1, 64], F32, tag="sq")
            for t in range(Tb):
                nc.tensor.matmul(
                    out=sum_ps[:, :], lhsT=ones[:, :], rhs=out_sb[:, t, :],
                    start=(t == 0), stop=(t == Tb - 1),
                )
            sqt = bnspool.tile([128, 64], F32, tag="sqt")
            for t in range(Tb):
                nc.vector.tensor_tensor(
                    out=sqt[:, :], in0=out_sb[:, t, :], in1=out_sb[:, t, :],
                    op=mybir.AluOpType.mult)
                nc.tensor.matmul(
                    out=sq_ps[:, :], lhsT=ones[:, :], rhs=sqt[:, :],
                    start=(t == 0), stop=(t == Tb - 1),
                )
            st0 = bnspool.tile([1, 64], F32, tag="st0")
            st1 = bnspool.tile([1, 64], F32, tag="st1")
            nc.vector.tensor_copy(out=st0[:, :], in_=sum_ps[:, :])
            nc.vector.tensor_copy(out=st1[:, :], in_=sq_ps[:, :])
            nc.sync.dma_start(out=cc_in[0:1, :], in_=st0[:, :])
            nc.sync.dma_start(out=cc_in[1:2, :], in_=st1[:, :])
            nc.gpsimd.collective_compute(
                "AllReduce",
                mybir.AluOpType.add,
                ins=[cc_in[:, :]],
                outs=[cc_out[:, :]],
                replica_groups=[list(range(n_cores))],
            )
            gs0 = bnspool.tile([1, 64], F32, tag="gs0")
            gs1 = bnspool.tile([1, 64], F32, tag="gs1")
            nc.sync.dma_start(out=gs0[:, :], in_=cc_out[0:1, :])
            nc.sync.dma_start(out=gs1[:, :], in_=cc_out[1:2, :])
            gam_t = bnspool.tile([1, 64], F32, tag="gam")
            bet_t = bnspool.tile([1, 64], F32, tag="bet")
            nc.sync.dma_start(out=gam_t[:, :], in_=gb[0:1, :])
            nc.sync.dma_start(out=bet_t[:, :], in_=gb[1:2, :])

            inv_n = 1.0 / float(n_out)
            mean_t = bnspool.tile([1, 64], F32, tag="mean")
            ex2_t = bnspool.tile([1, 64], F32, tag="ex2")
            var_t = bnspool.tile([1, 64], F32, tag="var")
            sd_t = bnspool.tile([1, 64], F32, tag="sd")
            rs_t = bnspool.tile([1, 64], F32, tag="rs")
            a_t = bnspool.tile([1, 64], F32, tag="a")
            b_t = bnspool.tile([1, 64], F32, tag="b")
            nc.vector.tensor_scalar_mul(mean_t[:, :], gs0[:, :], inv_n)
            nc.vector.tensor_scalar_mul(ex2_t[:, :], gs1[:, :], inv_n)
            nc.vector.tensor_tensor(
                out=var_t[:, :], in0=mean_t[:, :], in1=mean_t[:, :],
                op=mybir.AluOpType.mult)
            nc.vector.tensor_tensor(
                out=var_t[:, :], in0=ex2_t[:, :], in1=var_t[:, :],
                op=mybir.AluOpType.subtract)
            nc.vector.tensor_scalar_add(var_t[:, :], var_t[:, :], BN_EPS)
            nc.scalar.activation(
                out=sd_t[:, :], in_=var_t[:, :],
                func=mybir.ActivationFunctionType.Sqrt)
            nc.vector.reciprocal(out=rs_t[:, :], in_=sd_t[:, :])
            nc.vector.tensor_tensor(
                out=a_t[:, :], in0=gam_t[:, :], in1=rs_t[:, :],
                op=mybir.AluOpType.mult)
            nc.vector.tensor_tensor(
                out=b_t[:, :], in0=mean_t[:, :], in1=a_t[:, :],
                op=mybir.AluOpType.mult)
            nc.vector.tensor_tensor(
                out=b_t[:, :], in0=bet_t[:, :], in1=b_t[:, :],
                op=mybir.AluOpType.subtract)
            # broadcast [1,64] -> [128,64] via PE (ones[1,128]^T @ row)
            ones_row = bnspool.tile([1, 128], F32, tag="ones_row")
            nc.vector.memset(ones_row[:, :], 1.0)
            a_full = bnspool.tile([128, 64], F32, tag="afull")
            b_full = bnspool.tile([128, 64], F32, tag="bfull")
            ab_ps = bnps.tile([128, 64], F32, tag="abps")
            nc.tensor.matmul(
                out=ab_ps[:, :], lhsT=ones_row[:, :], rhs=a_t[:, :],
                start=True, stop=True)
            nc.vector.tensor_copy(out=a_full[:, :], in_=ab_ps[:, :])
            nc.tensor.matmul(
                out=ab_ps[:, :], lhsT=ones_row[:, :], rhs=b_t[:, :],
                start=True, stop=True)
            nc.vector.tensor_copy(out=b_full[:, :], in_=ab_ps[:, :])
            for t in range(Tb):
                nc.vector.tensor_tensor(
                    out=out_sb[:, t, :], in0=out_sb[:, t, :], in1=a_full[:, :],
                    op=mybir.AluOpType.mult)
                nc.vector.tensor_tensor(
                    out=out_sb[:, t, :], in0=out_sb[:, t, :], in1=b_full[:, :],
                    op=mybir.AluOpType.add)
                nc.scalar.activation(
                    out=out_sb[:, t, :], in_=out_sb[:, t, :],
                    func=mybir.ActivationFunctionType.Relu)
            nc.sync.dma_start(out=y[:, :], in_=out_sb[:, :, :])

    nc.compile()
    return nc


def _prepare(feats, W, gamma, beta, in_map, out_map, n_out, n_cores, dup_safe,
             expand=1):
    """Host prep shared by kernel() and tests. Returns (nc, in_maps, plan)."""
    n_out = int(n_out)
    K, Cin, Cout = W.shape
    assert Cin == 64 and Cout == 64
    in_map = np.asarray(in_map, dtype=np.int64)
    out_map = np.asarray(out_map, dtype=np.int64)
    feats = np.asarray(feats, dtype=np.float32)
    W = np.asarray(W, dtype=np.float32)

    plan, gidx_all, sidx_all = _route(
        in_map, out_map, n_out, n_cores, dup_safe, expand)

    ftab_rows = _roundup(feats.shape[0], CHUNK)
    ftab = np.zeros((ftab_rows, 128), dtype=ml_dtypes.bfloat16)
    ftab[:feats.shape[0], :64] = feats.astype(ml_dtypes.bfloat16)

    # W padded: [128 ic, K*64] bf16, rows 64..127 zero
    wt = np.zeros((128, K * 64), dtype=ml_dtypes.bfloat16)
    wt[:64, :] = (
        W.transpose(1, 0, 2).reshape(64, K * 64).astype(ml_dtypes.bfloat16))

    gb = np.stack([np.asarray(gamma, np.float32),
                   np.asarray(beta, np.float32)])

    nc = _build(plan, n_out, ftab_rows, n_cores)
    in_maps = [
        dict(ftab=ftab, wt=wt, gidx=gidx_all[c], sidx=sidx_all[c], gb=gb)
        for c in range(n_cores)
    ]
    return nc, in_maps, plan


def kernel(feats, W, gamma, beta, in_map, out_map, n_out):
    from concourse.bass_utils import run_bass_kernel_spmd

    n_cores = 8
    dup_safe = os.environ.get("DECONV_DUP_SAFE", "0") == "1"
    expand = int(os.environ.get("DECONV_EXPAND", "1"))
    nc, in_maps, plan = _prepare(
        feats, W, gamma, beta, in_map, out_map, n_out, n_cores, dup_safe,
        expand)
    res = run_bass_kernel_spmd(nc, in_maps, list(range(n_cores)))
    rows = plan["rows_per_core"]
    out = np.concatenate(
        [res.results[c]["y"][:rows] for c in range(n_cores)], axis=0)
    return out.astype(np.float32)



# revision 11
# speedup vs baseline: 1.7165x; 1.7165x over previous
"""Trainium2 Bass kernel for nn_BasicDeconvolutionBlock.

Reference computation:
    gathered = feats[in_map]                         # [K, M, Cin]
    contrib  = einsum('kmc,kcd->kmd', gathered, W)   # [K, M, Cout]
    out      = zeros([n_out, Cout]).at[out_map].add(contrib)
    y        = relu(batchnorm(out))                  # batch stats over n_out rows

Strategy (8 NeuronCores, SPMD):
  - Host routes each (k, m) pair to the core owning its output row
    (row blocks of n_out/8).  Per core ~169k pairs.
  - Slot stream per core: for each feats chunk (int16 gather range, 32768
    rows), pairs are laid out in (stage, k) groups with NO per-group
    padding (shared caps = max pair count over cores; ragged matmul
    pieces handle group boundaries).  The stream is cut into 896-slot
    windows (the SWDGE per-call index cap; 1024+ wedges the Q7 ucode).
    Each window is one dma_gather call AND one dma_scatter_add call.
  - Duplicate out-rows inside one scatter call race in hardware, so the
    host EVICTS same-(window,row) duplicates to overflow stages appended
    to the same chunk run (iterated until dup-free).  Cross-window dups
    are safe: same-bank windows serialize via Tile WAW deps, and the two
    HBM accumulator banks alternate by window parity so adjacent windows
    overlap.
  - Gather: feats pre-cast to bf16, padded to 128 channels (256B rows);
    dma_gather(transpose=True) yields channel-major G[128, slots].
  - GEMM: per window, ragged pieces (cut at k-group and 128-tile
    boundaries) matmul into ONE PSUM bank region [128, 448] f32
    (slot-major: partition=slot%128, col block=slot//128), then a single
    DVE copy converts to an fp16 slab.
  - Scatter: dma_scatter_add (CCE add, fp16, elem 64, row stride 256B)
    into acc banks [acc_rows+128, 128] fp16 (cols 64:128 unused pad to
    satisfy the 256B row-stride requirement); pad/hole/evicted slots go
    to a dump row beyond acc_rows.
  - BN: fold banks, ones-matmul row sums + sum of squares, [2,64]
    AllReduce across 8 cores, normalize + ReLU, write [25088, 64] f32.
"""

import numpy as np

import sys

sys.path.insert(0, "/opt/trn_rl_repo")

import ml_dtypes  # noqa: E402

from concourse import bacc, bass, mybir  # noqa: E402
import concourse.tile as tile  # noqa: E402

BN_EPS = 1e-5
CHUNK = 32768  # int16 gather index range per feats chunk
SEG = 896  # max indices per SWDGE call (1024+ wedges the Q7 ucode)
NBANKS = 2
F32 = mybir.dt.float32
FP16 = mybir.dt.float16
BF16 = mybir.dt.bfloat16
I16 = mybir.dt.int16


def _roundup(x, m):
    return (x + m - 1) // m * m


def _occ_rank(keys):
    """Rank of each element among equal values of `keys` (stable)."""
    order = np.argsort(keys, kind="stable")
    ks = keys[order]
    n = len(ks)
    first = np.ones(n, dtype=bool)
    if n:
        first[1:] = ks[1:] != ks[:-1]
    grp = np.maximum.accumulate(np.where(first, np.arange(n), 0))
    rank_sorted = np.arange(n) - grp
    out = np.empty(n, dtype=np.int64)
    out[order] = rank_sorted
    return out


def _route(in_map, out_map, n_out, n_cores):
    """Host-side routing.  Returns (plan, gidx_all, sidx_all).

    plan.windows: list of dicts (c, slot0, n, gcol0, pieces=[(k, off, len)])
    where slot0/gcol0 are global and off is window-relative.
    """
    K, M = in_map.shape
    rows_per_core = n_out // n_cores
    assert rows_per_core * n_cores == n_out
    acc_rows = _roundup(rows_per_core, 128)
    dump_row = acc_rows
    nchunk = _roundup(int(in_map.max()) + 1, CHUNK) // CHUNK

    k_idx = np.repeat(np.arange(K, dtype=np.int64), M)
    in_flat = in_map.ravel().astype(np.int64)
    out_flat = out_map.ravel().astype(np.int64)
    core_of = out_flat // rows_per_core
    row_local = out_flat % rows_per_core
    chunk_of = in_flat // CHUNK
    idx_local = in_flat % CHUNK

    # pending pairs per (core, chunk)
    pend = {}
    for ci in range(n_cores):
        sel = np.nonzero(core_of == ci)[0]
        for c in range(nchunk):
            s2 = sel[chunk_of[sel] == c]
            pend[(ci, c)] = (k_idx[s2], idx_local[s2], row_local[s2])

    # iterate stages per chunk until dup-free; build shared group layout
    run_kk = [[] for _ in range(nchunk)]  # per-chunk per-slot k (shared)
    run_len = np.zeros(nchunk, dtype=np.int64)
    # per (core, chunk): placed slot -> (gidx value, sidx value)
    placed = {key: ([], [], []) for key in pend}  # slots, gvals, svals

    for c in range(nchunk):
        stage = 0
        used = {ci: np.empty(0, dtype=np.int64) for ci in range(n_cores)}
        while True:
            counts = np.zeros((n_cores, K), dtype=np.int64)
            for ci in range(n_cores):
                kk, _, _ = pend[(ci, c)]
                if len(kk):
                    np.add.at(counts[ci], kk, 1)
            caps = counts.max(axis=0)
            # 64-align group sizes so matmul piece starts land on PSUM base
            # partitions 0/64 (PE tile_position + AP base_partition limits)
            caps = (caps + 63) // 64 * 64
            if caps.sum() == 0:
                break
            assert stage < 40, "eviction did not converge"
            if stage >= 2:
                # late stages start on a fresh window so high-multiplicity
                # rows always find a free (window,row) slot -> convergence
                tgt = _roundup(int(run_len[c]), SEG)
                if tgt > run_len[c]:
                    lastk = run_kk[c][-1] if run_kk[c] else 0
                    run_kk[c].extend([lastk] * int(tgt - run_len[c]))
                    run_len[c] = tgt
            g0 = np.zeros(K + 1, dtype=np.int64)
            g0[1:] = np.cumsum(caps)
            base = run_len[c]
            for k in range(K):
                run_kk[c].extend([k] * int(caps[k]))
            run_len[c] += caps.sum()
            for ci in range(n_cores):
                kk, iv, rv = pend[(ci, c)]
                if not len(kk):
                    pend[(ci, c)] = (kk, iv, rv)
                    continue
                # in-group order: (occ among same (k,row), row) to spread
                # a row's duplicates across the group's windows
                occ = _occ_rank(kk * rows_per_core + rv)
                order = np.lexsort((rv, occ, kk))
                kk, iv, rv = kk[order], iv[order], rv[order]
                starts = np.concatenate(
                    [[0], np.cumsum(np.bincount(kk, minlength=K))])
                rank = np.arange(len(kk)) - starts[kk]
                slot = base + g0[kk] + rank
                win = slot // SEG
                key = win * rows_per_core + rv
                # keep the first pair (in provisional order) per (win,row),
                # excluding (win,row) pairs already used by earlier stages
                fresh = ~np.isin(key, used[ci])
                order2 = np.lexsort((np.arange(len(key)), key))
                ks = key[order2]
                first = np.ones(len(ks), dtype=bool)
                if len(ks):
                    first[1:] = ks[1:] != ks[:-1]
                keep = np.zeros(len(key), dtype=bool)
                keep[order2[first]] = True
                keep &= fresh
                used[ci] = np.concatenate([used[ci], key[keep]])
                ps, pg, pv = placed[(ci, c)]
                ps.append(slot[keep])
                pg.append(iv[keep])
                pv.append(rv[keep])
                pend[(ci, c)] = (kk[~keep], iv[~keep], rv[~keep])
            stage += 1

    # pad runs to 128; build windows and global offsets
    run_pad = np.array([_roundup(int(x), 128) for x in run_len], dtype=np.int64)
    for c in range(nchunk):
        lastk = run_kk[c][-1] if run_kk[c] else 0
        run_kk[c].extend([lastk] * int(run_pad[c] - run_len[c]))
    run0 = np.zeros(nchunk + 1, dtype=np.int64)
    run0[1:] = np.cumsum(run_pad)
    total_slots = int(run0[-1])

    windows = []
    for c in range(nchunk):
        kkarr = np.array(run_kk[c], dtype=np.int64)
        off = 0
        while off < run_pad[c]:
            n = int(min(SEG, run_pad[c] - off))
            # pieces: boundaries at k-change and 128-tile cuts, then
            # subdivided to satisfy PE tile-position rules (base 0: len
            # <=128, base 64: len <=64, base 32/96: len <=32)
            pieces = []
            j = 0
            while j < n:
                k = int(kkarr[off + j])
                e = j + 1
                while e < n and kkarr[off + e] == k and e % 128 != 0:
                    e += 1
                p0, rem = j, e - j
                while rem > 0:
                    bp = p0 % 128
                    assert bp in (0, 64), (bp, j, e)
                    allowed = 128 if bp == 0 else 64
                    take = min(rem, allowed)
                    pieces.append((k, p0, take))
                    p0 += take
                    rem -= take
                j = e
            windows.append(
                dict(
                    c=c,
                    slot0=int(run0[c] + off),
                    n=n,
                    gcol0=int((run0[c] + off) // 16),
                    pieces=pieces,
                )
            )
            off += n

    # per-core packed idx arrays (16-wrapped, replicated to 128 partitions)
    gcols = total_slots // 16
    gidx_all = np.zeros((n_cores, 128, gcols), dtype=np.int16)
    sidx_all = np.empty((n_cores, 128, gcols), dtype=np.int16)
    for ci in range(n_cores):
        gvals = np.zeros(total_slots, dtype=np.int16)
        svals = np.full(total_slots, dump_row, dtype=np.int16)
        for c in range(nchunk):
            ps, pg, pv = placed[(ci, c)]
            if not ps:
                continue
            slots = np.concatenate(ps) + run0[c]
            gvals[slots] = np.concatenate(pg).astype(np.int16)
            svals[slots] = np.concatenate(pv).astype(np.int16)
        gidx_all[ci] = np.tile(gvals.reshape(-1, 16).T, (8, 1))
        sidx_all[ci] = np.tile(svals.reshape(-1, 16).T, (8, 1))

    plan = dict(
        nchunk=nchunk,
        rows_per_core=rows_per_core,
        acc_rows=acc_rows,
        acc_total=acc_rows + 128,
        dump_row=dump_row,
        windows=windows,
        total_slots=total_slots,
        gcols=gcols,
    )
    return plan, gidx_all, sidx_all


def _build(plan, n_out, ftab_rows, n_cores):
    """Trace the Bass program.  Returns nc."""
    nc = bacc.Bacc("TRN2", target_bir_lowering=False, debug=False)

    K = 27
    Cout = 64
    acc_rows, acc_total = plan["acc_rows"], plan["acc_total"]
    windows = plan["windows"]
    Tb = acc_rows // 128  # BN column tiles

    ftab = nc.dram_tensor("ftab", [ftab_rows, 128], BF16, kind="ExternalInput")
    wt = nc.dram_tensor("wt", [128, K * Cout], BF16, kind="ExternalInput")
    gidx = nc.dram_tensor("gidx", [128, plan["gcols"]], I16, kind="ExternalInput")
    sidx = nc.dram_tensor("sidx", [128, plan["gcols"]], I16, kind="ExternalInput")
    gb = nc.dram_tensor("gb", [2, Cout], F32, kind="ExternalInput")
    accs = [
        nc.dram_tensor(f"acc{b}", [acc_total, 128], FP16) for b in range(NBANKS)
    ]
    cc_in = nc.dram_tensor("cc_in", [2, Cout], F32)
    cc_out = nc.dram_tensor("cc_out", [2, Cout], F32, addr_space="Shared")
    y = nc.dram_tensor("y", [acc_rows, Cout], F32, kind="ExternalOutput")

    with tile.TileContext(nc) as tc:
        with (
            tc.tile_pool(name="const", bufs=1) as cpool,
            tc.tile_pool(name="gpool", bufs=4) as gpool,
            tc.tile_pool(name="slab", bufs=4) as slabpool,
            tc.tile_pool(name="gixp", bufs=4) as gixpool,
            tc.tile_pool(name="sixp", bufs=4) as sixpool,
            tc.tile_pool(name="psum", bufs=6, space="PSUM") as pspool,
        ):
            w_sb = cpool.tile([128, K * Cout], BF16, tag="w")
            nc.sync.dma_start(out=w_sb[:, :], in_=wt[:, :])
            zed = cpool.tile([128, 6400], FP16, tag="zed")
            nc.vector.memset(zed[:, :], 0.0)
            zrows = 128 * 6400 // 128  # 6400 rows per DMA
            for bank in accs:
                r0 = 0
                while r0 < acc_total:
                    rcnt = min(zrows, acc_total - r0)
                    nc.sync.dma_start(
                        out=bank[r0:r0 + rcnt, :],
                        in_=zed[:, :rcnt],
                    )
                    r0 += rcnt

            for wi, w in enumerate(windows):
                n = w["n"]
                ntile = n // 128
                gi = gixpool.tile([128, SEG // 16], I16, tag="gi")
                nc.sync.dma_start(
                    out=gi[:, :n // 16],
                    in_=gidx[:, w["gcol0"]:w["gcol0"] + n // 16],
                )
                g = gpool.tile([128, 1, SEG], BF16, tag="g")
                c = w["c"]
                nc.gpsimd.dma_gather(
                    out_ap=g[:, :, :n],
                    in_ap=ftab[c * CHUNK:min((c + 1) * CHUNK, ftab_rows), :],
                    idxs_ap=gi[:, :n // 16],
                    num_idxs=n,
                    num_idxs_reg=n,
                    elem_size=128,
                    transpose=True,
                )
                ps = pspool.tile([128, 8, 64], F32, tag="ps")
                for (k, off, ln) in w["pieces"]:
                    p0 = off % 128
                    t = off // 128
                    nc.tensor.matmul(
                        out=ps[p0:p0 + ln, t, :],
                        lhsT=g[:, 0, off:off + ln],
                        rhs=w_sb[:, k * Cout:(k + 1) * Cout],
                        start=True, stop=True,
                    )
                slab = slabpool.tile([128, SEG // 128, 64], FP16, tag="slab")
                nc.vector.tensor_copy(
                    out=slab[:, :ntile, :], in_=ps[:, :ntile, :])
                si = sixpool.tile([128, SEG // 16], I16, tag="si")
                nc.sync.dma_start(
                    out=si[:, :n // 16],
                    in_=sidx[:, w["gcol0"]:w["gcol0"] + n // 16],
                )
                nc.gpsimd.dma_scatter_add(
                    out_ap=accs[wi % NBANKS][:, 0:64],
                    in_ap=slab[:, :ntile, :],
                    idxs_ap=si[:, :n // 16],
                    num_idxs=n,
                    num_idxs_reg=n,
                    elem_size=64,
                    elem_step=128,
                )

        # ---- BN phase ----
        with (
            tc.tile_pool(name="bn", bufs=1) as bnpool,
            tc.tile_pool(name="bnb", bufs=1) as bnbpool,
            tc.tile_pool(name="bns", bufs=4) as bnspool,
            tc.tile_pool(name="bnp", bufs=2, space="PSUM") as bnps,
        ):
            out_sb = bnpool.tile([128, Tb, 64], F32, tag="outsb")
            tmp_sb = bnpool.tile([128, Tb, 64], F32, tag="tmpsb")
            for b in range(NBANKS):
                bank_sb = bnbpool.tile([128, Tb, 128], FP16, tag="bank")
                nc.sync.dma_start(out=bank_sb[:, :, :], in_=accs[b][0:acc_rows, :])
                if b == 0:
                    nc.vector.tensor_copy(
                        out=out_sb[:, :, :], in_=bank_sb[:, :, 0:64])
                else:
                    nc.vector.tensor_copy(
                        out=tmp_sb[:, :, :], in_=bank_sb[:, :, 0:64])
                    nc.vector.tensor_tensor(
                        out=out_sb[:, :, :], in0=out_sb[:, :, :],
                        in1=tmp_sb[:, :, :], op=mybir.AluOpType.add)
            ones = bnpool.tile([128, 1], F32, tag="ones")
            nc.vector.memset(ones[:, :], 1.0)
            sum_ps = bnps.tile([1, 64], F32, tag="sum")
            sq_ps = bnps.tile([1, 64], F32, tag="sq")
            for t in range(Tb):
                nc.tensor.matmul(
                    out=sum_ps[:, :], lhsT=ones[:, :], rhs=out_sb[:, t, :],
                    start=(t == 0), stop=(t == Tb - 1),
                )
            sqt = bnspool.tile([128, 64], F32, tag="sqt")
            for t in range(Tb):
                nc.vector.tensor_tensor(
                    out=sqt[:, :], in0=out_sb[:, t, :], in1=out_sb[:, t, :],
                    op=mybir.AluOpType.mult)
                nc.tensor.matmul(
                    out=sq_ps[:, :], lhsT=ones[:, :], rhs=sqt[:, :],
                    start=(t == 0), stop=(t == Tb - 1),
                )
            st0 = bnspool.tile([1, 64], F32, tag="st0")
            st1 = bnspool.tile([1, 64], F32, tag="st1")
            nc.vector.tensor_copy(out=st0[:, :], in_=sum_ps[:, :])
            nc.vector.tensor_copy(out=st1[:, :], in_=sq_ps[:, :])
            nc.sync.dma_start(out=cc_in[0:1, :], in_=st0[:, :])
            nc.sync.dma_start(out=cc_in[1:2, :], in_=st1[:, :])
            nc.gpsimd.collective_compute(
                "AllReduce",
                mybir.AluOpType.add,
                ins=[cc_in[:, :]],
                outs=[cc_out[:, :]],
                replica_groups=[list(range(n_cores))],
            )
            gs0 = bnspool.tile([1, 64], F32, tag="gs0")
            gs1 = bnspool.tile([1, 64], F32, tag="gs1")
            nc.sync.dma_start(out=gs0[:, :], in_=cc_out[0:1, :])
            nc.sync.dma_start(out=gs1[:, :], in_=cc_out[1:2, :])
            gam_t = bnspool.tile([1, 64], F32, tag="gam")
            bet_t = bnspool.tile([1, 64], F32, tag="bet")
            nc.sync.dma_start(out=gam_t[:, :], in_=gb[0:1, :])
            nc.sync.dma_start(out=bet_t[:, :], in_=gb[1:2, :])

            inv_n = 1.0 / float(n_out)
            mean_t = bnspool.tile([1, 64], F32, tag="mean")
            ex2_t = bnspool.tile([1, 64], F32, tag="ex2")
            var_t = bnspool.tile([1, 64], F32, tag="var")
            sd_t = bnspool.tile([1, 64], F32, tag="sd")
            rs_t = bnspool.tile([1, 64], F32, tag="rs")
            a_t = bnspool.tile([1, 64], F32, tag="a")
            b_t = bnspool.tile([1, 64], F32, tag="b")
            nc.vector.tensor_scalar_mul(mean_t[:, :], gs0[:, :], inv_n)
            nc.vector.tensor_scalar_mul(ex2_t[:, :], gs1[:, :], inv_n)
            nc.vector.tensor_tensor(
                out=var_t[:, :], in0=mean_t[:, :], in1=mean_t[:, :],
                op=mybir.AluOpType.mult)
            nc.vector.tensor_tensor(
                out=var_t[:, :], in0=ex2_t[:, :], in1=var_t[:, :],
                op=mybir.AluOpType.subtract)
            nc.vector.tensor_scalar_add(var_t[:, :], var_t[:, :], BN_EPS)
            nc.scalar.activation(
                out=sd_t[:, :], in_=var_t[:, :],
                func=mybir.ActivationFunctionType.Sqrt)
            nc.vector.reciprocal(out=rs_t[:, :], in_=sd_t[:, :])
            nc.vector.tensor_tensor(
                out=a_t[:, :], in0=gam_t[:, :], in1=rs_t[:, :],
                op=mybir.AluOpType.mult)
            nc.vector.tensor_tensor(
                out=b_t[:, :], in0=mean_t[:, :], in1=a_t[:, :],
                op=mybir.AluOpType.mult)
            nc.vector.tensor_tensor(
                out=b_t[:, :], in0=bet_t[:, :], in1=b_t[:, :],
                op=mybir.AluOpType.subtract)
            # broadcast [1,64] -> [128,64] via PE (ones[128,1] @ row)
            ones_row = bnspool.tile([1, 128], F32, tag="ones_row")
            nc.vector.memset(ones_row[:, :], 1.0)
            a_full = bnspool.tile([128, 64], F32, tag="afull")
            b_full = bnspool.tile([128, 64], F32, tag="bfull")
            ab_ps = bnps.tile([128, 64], F32, tag="abps")
            nc.tensor.matmul(
                out=ab_ps[:, :], lhsT=ones_row[:, :], rhs=a_t[:, :],
                start=True, stop=True)
            nc.vector.tensor_copy(out=a_full[:, :], in_=ab_ps[:, :])
            nc.tensor.matmul(
                out=ab_ps[:, :], lhsT=ones_row[:, :], rhs=b_t[:, :],
                start=True, stop=True)
            nc.vector.tensor_copy(out=b_full[:, :], in_=ab_ps[:, :])
            for t in range(Tb):
                nc.vector.tensor_tensor(
                    out=out_sb[:, t, :], in0=out_sb[:, t, :], in1=a_full[:, :],
                    op=mybir.AluOpType.mult)
                nc.vector.tensor_tensor(
                    out=out_sb[:, t, :], in0=out_sb[:, t, :], in1=b_full[:, :],
                    op=mybir.AluOpType.add)
                nc.scalar.activation(
                    out=out_sb[:, t, :], in_=out_sb[:, t, :],
                    func=mybir.ActivationFunctionType.Relu)
            nc.sync.dma_start(out=y[:, :], in_=out_sb[:, :, :])

    nc.compile()
    return nc


def _prepare(feats, W, gamma, beta, in_map, out_map, n_out, n_cores):
    """Host prep shared by kernel() and tests.  Returns (nc, in_maps, plan)."""
    n_out = int(n_out)
    K, Cin, Cout = W.shape
    assert Cin == 64 and Cout == 64
    in_map = np.asarray(in_map, dtype=np.int64)
    out_map = np.asarray(out_map, dtype=np.int64)
    feats = np.asarray(feats, dtype=np.float32)
    W = np.asarray(W, dtype=np.float32)

    plan, gidx_all, sidx_all = _route(in_map, out_map, n_out, n_cores)

    ftab_rows = _roundup(feats.shape[0], CHUNK)
    ftab = np.zeros((ftab_rows, 128), dtype=ml_dtypes.bfloat16)
    ftab[:feats.shape[0], :64] = feats.astype(ml_dtypes.bfloat16)

    wt = np.zeros((128, K * 64), dtype=ml_dtypes.bfloat16)
    wt[:64, :] = (
        W.transpose(1, 0, 2).reshape(64, K * 64).astype(ml_dtypes.bfloat16))

    gb = np.stack([np.asarray(gamma, np.float32),
                   np.asarray(beta, np.float32)])

    nc = _build(plan, n_out, ftab_rows, n_cores)
    in_maps = [
        dict(ftab=ftab, wt=wt, gidx=gidx_all[c], sidx=sidx_all[c], gb=gb)
        for c in range(n_cores)
    ]
    return nc, in_maps, plan


def kernel(feats, W, gamma, beta, in_map, out_map, n_out):
    from concourse.bass_utils import run_bass_kernel_spmd

    n_cores = 8
    nc, in_maps, plan = _prepare(
        feats, W, gamma, beta, in_map, out_map, n_out, n_cores)
    res = run_bass_kernel_spmd(nc, in_maps, list(range(n_cores)))
    rows = plan["rows_per_core"]
    out = np.concatenate(
        [res.results[c]["y"][:rows] for c in range(n_cores)], axis=0)
    return out.astype(np.float32)


# revision 22
# speedup vs baseline: 1.9716x; 1.1486x over previous
"""Trainium2 Bass kernel for nn_BasicDeconvolutionBlock.

Reference computation:
    gathered = feats[in_map]                         # [K, M, Cin]
    contrib  = einsum('kmc,kcd->kmd', gathered, W)   # [K, M, Cout]
    out      = zeros([n_out, Cout]).at[out_map].add(contrib)
    y        = relu(batchnorm(out))                  # batch stats over n_out rows

Strategy (8 NeuronCores, SPMD):
  - Host routes each (k, m) pair to the core owning its output row
    (row blocks of n_out/8).  Per core ~169k pairs.
  - Slot stream per core: for each feats chunk (int16 gather range, 32768
    rows), pairs are laid out in (stage, k) groups with NO per-group
    padding (shared caps = max pair count over cores; ragged matmul
    pieces handle group boundaries).  The stream is cut into 896-slot
    windows (the SWDGE per-call index cap; 1024+ wedges the Q7 ucode).
    Each window is one dma_gather call AND one dma_scatter_add call.
  - Duplicate out-rows inside one scatter call race in hardware, so the
    host EVICTS same-(window,row) duplicates to overflow stages appended
    to the same chunk run (iterated until dup-free).  Cross-window dups
    are safe: same-bank windows serialize via Tile WAW deps, and the two
    HBM accumulator banks alternate by window parity so adjacent windows
    overlap.
  - Gather: feats pre-cast to bf16, padded to 128 channels (256B rows);
    dma_gather(transpose=True) yields channel-major G[128, slots].
  - GEMM: per window, ragged pieces (cut at k-group and 128-tile
    boundaries) matmul into ONE PSUM bank region [128, 448] f32
    (slot-major: partition=slot%128, col block=slot//128), then a single
    DVE copy converts to an fp16 slab.
  - Scatter: dma_scatter_add (CCE add, fp16, elem 64, row stride 256B)
    into acc banks [acc_rows+128, 128] fp16 (cols 64:128 unused pad to
    satisfy the 256B row-stride requirement); pad/hole/evicted slots go
    to a dump row beyond acc_rows.
  - BN: fold banks, ones-matmul row sums + sum of squares, [2,64]
    AllReduce across 8 cores, normalize + ReLU, write [25088, 64] f32.
"""

import numpy as np

import sys

sys.path.insert(0, "/opt/trn_rl_repo")

import ml_dtypes  # noqa: E402

from concourse import bacc, bass, mybir  # noqa: E402
import concourse.tile as tile  # noqa: E402

BN_EPS = 1e-5
CHUNK = 32768  # int16 gather index range per feats chunk
SEG = 896  # max indices per SWDGE call (1024+ wedges the Q7 ucode)
NBANKS = 2
F32 = mybir.dt.float32
FP16 = mybir.dt.float16
BF16 = mybir.dt.bfloat16
I16 = mybir.dt.int16


def _roundup(x, m):
    return (x + m - 1) // m * m


def _occ_rank(keys):
    """Rank of each element among equal values of `keys` (stable)."""
    order = np.argsort(keys, kind="stable")
    ks = keys[order]
    n = len(ks)
    first = np.ones(n, dtype=bool)
    if n:
        first[1:] = ks[1:] != ks[:-1]
    grp = np.maximum.accumulate(np.where(first, np.arange(n), 0))
    rank_sorted = np.arange(n) - grp
    out = np.empty(n, dtype=np.int64)
    out[order] = rank_sorted
    return out


def _route(in_map, out_map, n_out, n_cores):
    """Host-side routing.  Returns (plan, gidx_all, sidx_all).

    plan.windows: list of dicts (c, slot0, n, gcol0, pieces=[(k, off, len)])
    where slot0/gcol0 are global and off is window-relative.
    """
    K, M = in_map.shape
    rows_per_core = n_out // n_cores
    assert rows_per_core * n_cores == n_out
    acc_rows = _roundup(rows_per_core, 128)
    dump_row = acc_rows
    nchunk = _roundup(int(in_map.max()) + 1, CHUNK) // CHUNK

    k_idx = np.repeat(np.arange(K, dtype=np.int64), M)
    in_flat = in_map.ravel().astype(np.int64)
    out_flat = out_map.ravel().astype(np.int64)
    core_of = out_flat // rows_per_core
    row_local = out_flat % rows_per_core
    chunk_of = in_flat // CHUNK
    idx_local = in_flat % CHUNK

    # pending pairs per (core, chunk)
    pend = {}
    for ci in range(n_cores):
        sel = np.nonzero(core_of == ci)[0]
        for c in range(nchunk):
            s2 = sel[chunk_of[sel] == c]
            pend[(ci, c)] = (k_idx[s2], idx_local[s2], row_local[s2])

    # iterate stages per chunk until dup-free; build shared group layout
    run_kk = [[] for _ in range(nchunk)]  # per-chunk per-slot k (shared)
    run_len = np.zeros(nchunk, dtype=np.int64)
    # per (core, chunk): placed slot -> (gidx value, sidx value)
    placed = {key: ([], [], []) for key in pend}  # slots, gvals, svals

    for c in range(nchunk):
        stage = 0
        used = {ci: np.empty(0, dtype=np.int64) for ci in range(n_cores)}
        while True:
            counts = np.zeros((n_cores, K), dtype=np.int64)
            for ci in range(n_cores):
                kk, _, _ = pend[(ci, c)]
                if len(kk):
                    np.add.at(counts[ci], kk, 1)
            caps = counts.max(axis=0)
            # 64-align group sizes so matmul piece starts land on PSUM base
            # partitions 0/64 (PE tile_position + AP base_partition limits)
            caps = (caps + 63) // 64 * 64
            if caps.sum() == 0:
                break
            assert stage < 40, "eviction did not converge"
            if stage >= 2:
                # late stages start on a fresh window so high-multiplicity
                # rows always find a free (window,row) slot -> convergence
                tgt = _roundup(int(run_len[c]), SEG)
                if tgt > run_len[c]:
                    lastk = run_kk[c][-1] if run_kk[c] else 0
                    run_kk[c].extend([lastk] * int(tgt - run_len[c]))
                    run_len[c] = tgt
            g0 = np.zeros(K + 1, dtype=np.int64)
            g0[1:] = np.cumsum(caps)
            base = run_len[c]
            for k in range(K):
                run_kk[c].extend([k] * int(caps[k]))
            run_len[c] += caps.sum()
            for ci in range(n_cores):
                kk, iv, rv = pend[(ci, c)]
                if not len(kk):
                    pend[(ci, c)] = (kk, iv, rv)
                    continue
                # in-group order: (occ among same (k,row), row) to spread
                # a row's duplicates across the group's windows
                occ = _occ_rank(kk * rows_per_core + rv)
                order = np.lexsort((rv, occ, kk))
                kk, iv, rv = kk[order], iv[order], rv[order]
                starts = np.concatenate(
                    [[0], np.cumsum(np.bincount(kk, minlength=K))])
                rank = np.arange(len(kk)) - starts[kk]
                slot = base + g0[kk] + rank
                win = slot // SEG
                key = win * rows_per_core + rv
                # keep the first pair (in provisional order) per (win,row),
                # excluding (win,row) pairs already used by earlier stages
                fresh = ~np.isin(key, used[ci])
                order2 = np.lexsort((np.arange(len(key)), key))
                ks = key[order2]
                first = np.ones(len(ks), dtype=bool)
                if len(ks):
                    first[1:] = ks[1:] != ks[:-1]
                keep = np.zeros(len(key), dtype=bool)
                keep[order2[first]] = True
                keep &= fresh
                used[ci] = np.concatenate([used[ci], key[keep]])
                ps, pg, pv = placed[(ci, c)]
                ps.append(slot[keep])
                pg.append(iv[keep])
                pv.append(rv[keep])
                pend[(ci, c)] = (kk[~keep], iv[~keep], rv[~keep])
            stage += 1

    # pad runs to 128; build windows and global offsets
    run_pad = np.array([_roundup(int(x), 128) for x in run_len], dtype=np.int64)
    for c in range(nchunk):
        lastk = run_kk[c][-1] if run_kk[c] else 0
        run_kk[c].extend([lastk] * int(run_pad[c] - run_len[c]))
    run0 = np.zeros(nchunk + 1, dtype=np.int64)
    run0[1:] = np.cumsum(run_pad)
    total_slots = int(run0[-1])

    windows = []
    for c in range(nchunk):
        kkarr = np.array(run_kk[c], dtype=np.int64)
        off = 0
        while off < run_pad[c]:
            n = int(min(SEG, run_pad[c] - off))
            # pieces: boundaries at k-change and 128-tile cuts, then
            # subdivided to satisfy PE tile-position rules (base 0: len
            # <=128, base 64: len <=64, base 32/96: len <=32)
            pieces = []
            j = 0
            while j < n:
                k = int(kkarr[off + j])
                e = j + 1
                while e < n and kkarr[off + e] == k and e % 128 != 0:
                    e += 1
                p0, rem = j, e - j
                while rem > 0:
                    bp = p0 % 128
                    assert bp in (0, 64), (bp, j, e)
                    allowed = 128 if bp == 0 else 64
                    take = min(rem, allowed)
                    pieces.append((k, p0, take))
                    p0 += take
                    rem -= take
                j = e
            windows.append(
                dict(
                    c=c,
                    slot0=int(run0[c] + off),
                    n=n,
                    gcol0=int((run0[c] + off) // 16),
                    pieces=pieces,
                )
            )
            off += n

    # per-core packed idx arrays (16-wrapped, replicated to 128 partitions);
    # gather and scatter indices for each window are interleaved into ONE
    # tensor so a single DMA per window loads both
    gcols = total_slots // 16
    gsidx_all = np.zeros((n_cores, 128, 2 * gcols), dtype=np.int16)
    for ci in range(n_cores):
        gvals = np.zeros(total_slots, dtype=np.int16)
        svals = np.full(total_slots, dump_row, dtype=np.int16)
        for c in range(nchunk):
            ps, pg, pv = placed[(ci, c)]
            if not ps:
                continue
            slots = np.concatenate(ps) + run0[c]
            gvals[slots] = np.concatenate(pg).astype(np.int16)
            svals[slots] = np.concatenate(pv).astype(np.int16)
        gwrap = np.tile(gvals.reshape(-1, 16).T, (8, 1))
        swrap = np.tile(svals.reshape(-1, 16).T, (8, 1))
        for w in windows:
            c0, nw = w["gcol0"], w["n"] // 16
            gsidx_all[ci][:, 2 * c0:2 * c0 + nw] = gwrap[:, c0:c0 + nw]
            gsidx_all[ci][:, 2 * c0 + nw:2 * c0 + 2 * nw] = (
                swrap[:, c0:c0 + nw])

    plan = dict(
        nchunk=nchunk,
        rows_per_core=rows_per_core,
        acc_rows=acc_rows,
        acc_total=acc_rows + 128,
        dump_row=dump_row,
        windows=windows,
        total_slots=total_slots,
        gcols=gcols,
    )
    return plan, gidx_all, sidx_all


def _build(plan, n_out, ftab_rows, n_cores):
    """Trace the Bass program.  Returns nc."""
    nc = bacc.Bacc("TRN2", target_bir_lowering=False, debug=False)

    K = 27
    Cout = 64
    acc_rows, acc_total = plan["acc_rows"], plan["acc_total"]
    windows = plan["windows"]
    Tb = acc_rows // 128  # BN column tiles

    ftab = nc.dram_tensor("ftab", [ftab_rows, 128], BF16, kind="ExternalInput")
    wt = nc.dram_tensor("wt", [128, K * Cout], BF16, kind="ExternalInput")
    gidx = nc.dram_tensor("gidx", [128, plan["gcols"]], I16, kind="ExternalInput")
    sidx = nc.dram_tensor("sidx", [128, plan["gcols"]], I16, kind="ExternalInput")
    gb = nc.dram_tensor("gb", [2, Cout], F32, kind="ExternalInput")
    accs = [
        nc.dram_tensor(f"acc{b}", [acc_total, 128], FP16) for b in range(NBANKS)
    ]
    cc_in = nc.dram_tensor("cc_in", [2, Cout], F32)
    cc_out = nc.dram_tensor("cc_out", [2, Cout], F32, addr_space="Shared")
    # 3D view of the [acc_rows, 64] output (row = p*Tb + t) so row-half
    # writes can overlap the normalize pipeline
    y = nc.dram_tensor("y", [128, acc_rows // 128, Cout], F32,
                       kind="ExternalOutput")

    with tile.TileContext(nc) as tc:
        with (
            tc.tile_pool(name="const", bufs=1) as cpool,
            tc.tile_pool(name="gpool", bufs=4) as gpool,
            tc.tile_pool(name="slab", bufs=4) as slabpool,
            tc.tile_pool(name="gixp", bufs=4) as gixpool,
            tc.tile_pool(name="sixp", bufs=4) as sixpool,
            tc.tile_pool(name="psum", bufs=6, space="PSUM") as pspool,
        ):
            w_sb = cpool.tile([128, K * Cout], BF16, tag="w")
            nc.sync.dma_start(out=w_sb[:, :], in_=wt[:, :])
            zed = cpool.tile([128, 6400], FP16, tag="zed")
            nc.vector.memset(zed[:, :], 0.0)
            zrows = 128 * 6400 // 128  # 6400 rows per DMA
            for bank in accs:
                r0 = 0
                while r0 < acc_total:
                    rcnt = min(zrows, acc_total - r0)
                    nc.sync.dma_start(
                        out=bank[r0:r0 + rcnt, :],
                        in_=zed[:, :rcnt],
                    )
                    r0 += rcnt

            for wi, w in enumerate(windows):
                n = w["n"]
                ntile = n // 128
                gi = gixpool.tile([128, SEG // 16], I16, tag="gi")
                nc.sync.dma_start(
                    out=gi[:, :n // 16],
                    in_=gidx[:, w["gcol0"]:w["gcol0"] + n // 16],
                )
                g = gpool.tile([128, 1, SEG], BF16, tag="g")
                c = w["c"]
                nc.gpsimd.dma_gather(
                    out_ap=g[:, :, :n],
                    in_ap=ftab[c * CHUNK:min((c + 1) * CHUNK, ftab_rows), :],
                    idxs_ap=gi[:, :n // 16],
                    num_idxs=n,
                    num_idxs_reg=n,
                    elem_size=128,
                    transpose=True,
                )
                ps = pspool.tile([128, 8, 64], F32, tag="ps")
                for (k, off, ln) in w["pieces"]:
                    p0 = off % 128
                    t = off // 128
                    nc.tensor.matmul(
                        out=ps[p0:p0 + ln, t, :],
                        lhsT=g[:, 0, off:off + ln],
                        rhs=w_sb[:, k * Cout:(k + 1) * Cout],
                        start=True, stop=True,
                    )
                slab = slabpool.tile([128, SEG // 128, 64], FP16, tag="slab")
                nc.vector.tensor_copy(
                    out=slab[:, :ntile, :], in_=ps[:, :ntile, :])
                si = sixpool.tile([128, SEG // 16], I16, tag="si")
                nc.sync.dma_start(
                    out=si[:, :n // 16],
                    in_=sidx[:, w["gcol0"]:w["gcol0"] + n // 16],
                )
                nc.gpsimd.dma_scatter_add(
                    out_ap=accs[wi % NBANKS][:, 0:64],
                    in_ap=slab[:, :ntile, :],
                    idxs_ap=si[:, :n // 16],
                    num_idxs=n,
                    num_idxs_reg=n,
                    elem_size=64,
                    elem_step=128,
                )

        # ---- BN phase ----
        Tp = _roundup(Tb, 8)  # stat tiles padded so all matmuls are 512-wide
        F32R = mybir.dt.float32r
        with (
            tc.tile_pool(name="bn", bufs=1) as bnpool,
            tc.tile_pool(name="bns", bufs=4) as bnspool,
            tc.tile_pool(name="bnp", bufs=2, space="PSUM") as bnps,
        ):
            out_sb = bnpool.tile([128, Tp, 64], F32, tag="outsb")
            gam_t = bnspool.tile([1, 64], F32, tag="gam")
            bet_t = bnspool.tile([1, 64], F32, tag="bet")
            nc.sync.dma_start(out=gam_t[:, :], in_=gb[0:1, :])
            nc.sync.dma_start(out=bet_t[:, :], in_=gb[1:2, :])
            with tc.tile_pool(name="bnb", bufs=2) as bnbpool:
                b0 = bnbpool.tile([128, Tb, 128], FP16, tag="bank")
                nc.sync.dma_start(out=b0[:, :, :], in_=accs[0][0:acc_rows, :])
                b1 = bnbpool.tile([128, Tb, 128], FP16, tag="bank")
                nc.sync.dma_start(out=b1[:, :, :], in_=accs[1][0:acc_rows, :])
                if Tp > Tb:
                    nc.vector.memset(out_sb[:, Tb:Tp, :], 0.0)
                nc.vector.tensor_tensor(
                    out=out_sb[:, 0:Tb, :], in0=b0[:, :, 0:64],
                    in1=b1[:, :, 0:64], op=mybir.AluOpType.add)
            ones = bnpool.tile([128, 1], BF16, tag="ones")
            nc.vector.memset(ones[:, :], 1.0)
            sum_ps = bnps.tile([1, 512], F32, tag="sum")
            sq_ps = bnps.tile([1, 512], F32, tag="sq")
            ngrp = Tp // 8
            for i in range(ngrp):
                xbt = bnspool.tile([128, 8, 64], BF16, tag="xbt")
                nc.vector.tensor_copy(
                    out=xbt[:, :, :], in_=out_sb[:, 8 * i:8 * i + 8, :])
                nc.tensor.matmul(
                    out=sum_ps[:, :], lhsT=ones[:, :], rhs=xbt[:, :, :],
                    start=(i == 0), stop=(i == ngrp - 1),
                    skip_group_check=True,
                )
                sqt = bnspool.tile([128, 8, 64], BF16, tag="sqt")
                nc.vector.tensor_tensor(
                    out=sqt[:, :, :], in0=xbt[:, :, :], in1=xbt[:, :, :],
                    op=mybir.AluOpType.mult)
                nc.tensor.matmul(
                    out=sq_ps[:, :], lhsT=ones[:, :], rhs=sqt[:, :, :],
                    start=(i == 0), stop=(i == ngrp - 1),
                    skip_group_check=True,
                )
            st0 = bnspool.tile([1, 512], F32, tag="st0")
            st1 = bnspool.tile([1, 512], F32, tag="st1")
            nc.vector.tensor_copy(out=st0[:, :], in_=sum_ps[:, :])
            nc.vector.tensor_copy(out=st1[:, :], in_=sq_ps[:, :])
            for st in (st0, st1):
                nc.vector.tensor_tensor(
                    out=st[:, 0:256], in0=st[:, 0:256], in1=st[:, 256:512],
                    op=mybir.AluOpType.add)
                nc.vector.tensor_tensor(
                    out=st[:, 0:128], in0=st[:, 0:128], in1=st[:, 128:256],
                    op=mybir.AluOpType.add)
                nc.vector.tensor_tensor(
                    out=st[:, 0:64], in0=st[:, 0:64], in1=st[:, 64:128],
                    op=mybir.AluOpType.add)
            nc.sync.dma_start(out=cc_in[0:1, :], in_=st0[:, 0:64])
            nc.sync.dma_start(out=cc_in[1:2, :], in_=st1[:, 0:64])
            nc.gpsimd.collective_compute(
                "AllReduce",
                mybir.AluOpType.add,
                ins=[cc_in[:, :]],
                outs=[cc_out[:, :]],
                replica_groups=[list(range(n_cores))],
            )
            gs0 = bnspool.tile([1, 64], F32, tag="gs0")
            gs1 = bnspool.tile([1, 64], F32, tag="gs1")
            nc.sync.dma_start(out=gs0[:, :], in_=cc_out[0:1, :])
            nc.sync.dma_start(out=gs1[:, :], in_=cc_out[1:2, :])

            inv_n = 1.0 / float(n_out)
            mean_t = bnspool.tile([1, 64], F32, tag="mean")
            ex2_t = bnspool.tile([1, 64], F32, tag="ex2")
            var_t = bnspool.tile([1, 64], F32, tag="var")
            sd_t = bnspool.tile([1, 64], F32, tag="sd")
            rs_t = bnspool.tile([1, 64], F32, tag="rs")
            a_t = bnspool.tile([1, 64], F32, tag="a")
            b_t = bnspool.tile([1, 64], F32, tag="b")
            nc.vector.tensor_scalar_mul(mean_t[:, :], gs0[:, :], inv_n)
            nc.vector.tensor_scalar_mul(ex2_t[:, :], gs1[:, :], inv_n)
            nc.vector.tensor_tensor(
                out=var_t[:, :], in0=mean_t[:, :], in1=mean_t[:, :],
                op=mybir.AluOpType.mult)
            nc.vector.tensor_tensor(
                out=var_t[:, :], in0=ex2_t[:, :], in1=var_t[:, :],
                op=mybir.AluOpType.subtract)
            nc.vector.tensor_scalar_add(var_t[:, :], var_t[:, :], BN_EPS)
            nc.scalar.activation(
                out=sd_t[:, :], in_=var_t[:, :],
                func=mybir.ActivationFunctionType.Sqrt)
            nc.vector.reciprocal(out=rs_t[:, :], in_=sd_t[:, :])
            nc.vector.tensor_tensor(
                out=a_t[:, :], in0=gam_t[:, :], in1=rs_t[:, :],
                op=mybir.AluOpType.mult)
            nc.vector.tensor_tensor(
                out=b_t[:, :], in0=mean_t[:, :], in1=a_t[:, :],
                op=mybir.AluOpType.mult)
            nc.vector.tensor_tensor(
                out=b_t[:, :], in0=bet_t[:, :], in1=b_t[:, :],
                op=mybir.AluOpType.subtract)
            # broadcast [1,64] -> [128,64] via PE (ones[128,1] @ row)
            ones_row = bnspool.tile([1, 128], F32, tag="ones_row")
            nc.vector.memset(ones_row[:, :], 1.0)
            a_full = bnspool.tile([128, 64], F32, tag="afull")
            b_full = bnspool.tile([128, 64], F32, tag="bfull")
            ab_ps = bnps.tile([128, 64], F32, tag="abps")
            nc.tensor.matmul(
                out=ab_ps[:, :], lhsT=ones_row[:, :], rhs=a_t[:, :],
                start=True, stop=True)
            nc.vector.tensor_copy(out=a_full[:, :], in_=ab_ps[:, :])
            nc.tensor.matmul(
                out=ab_ps[:, :], lhsT=ones_row[:, :], rhs=b_t[:, :],
                start=True, stop=True)
            nc.vector.tensor_copy(out=b_full[:, :], in_=ab_ps[:, :])
            # fused normalize + relu over row halves, each half's y write
            # overlapping the next half's compute
            half = Tb // 2
            for (t0, t1) in ((0, half), (half, Tb)):
                nt = t1 - t0
                nc.vector.tensor_tensor(
                    out=out_sb[:, t0:t1, :], in0=out_sb[:, t0:t1, :],
                    in1=a_full[:, None, :].to_broadcast((128, nt, 64)),
                    op=mybir.AluOpType.mult)
                nc.vector.tensor_tensor(
                    out=out_sb[:, t0:t1, :], in0=out_sb[:, t0:t1, :],
                    in1=b_full[:, None, :].to_broadcast((128, nt, 64)),
                    op=mybir.AluOpType.add)
                nc.scalar.activation(
                    out=out_sb[:, t0:t1, :], in_=out_sb[:, t0:t1, :],
                    func=mybir.ActivationFunctionType.Relu)
                nc.sync.dma_start(
                    out=y[:, t0:t1, :], in_=out_sb[:, t0:t1, :])

    nc.compile()
    return nc


def _prepare(feats, W, gamma, beta, in_map, out_map, n_out, n_cores):
    """Host prep shared by kernel() and tests.  Returns (nc, in_maps, plan)."""
    n_out = int(n_out)
    K, Cin, Cout = W.shape
    assert Cin == 64 and Cout == 64
    in_map = np.asarray(in_map, dtype=np.int64)
    out_map = np.asarray(out_map, dtype=np.int64)
    feats = np.asarray(feats, dtype=np.float32)
    W = np.asarray(W, dtype=np.float32)

    plan, gidx_all, sidx_all = _route(in_map, out_map, n_out, n_cores)

    ftab_rows = _roundup(feats.shape[0], CHUNK)
    ftab = np.zeros((ftab_rows, 128), dtype=ml_dtypes.bfloat16)
    ftab[:feats.shape[0], :64] = feats.astype(ml_dtypes.bfloat16)

    wt = np.zeros((128, K * 64), dtype=ml_dtypes.bfloat16)
    wt[:64, :] = (
        W.transpose(1, 0, 2).reshape(64, K * 64).astype(ml_dtypes.bfloat16))

    gb = np.stack([np.asarray(gamma, np.float32),
                   np.asarray(beta, np.float32)])

    nc = _build(plan, n_out, ftab_rows, n_cores)
    in_maps = [
        dict(ftab=ftab, wt=wt, gidx=gidx_all[c], sidx=sidx_all[c], gb=gb)
        for c in range(n_cores)
    ]
    return nc, in_maps, plan


def kernel(feats, W, gamma, beta, in_map, out_map, n_out):
    from concourse.bass_utils import run_bass_kernel_spmd

    n_cores = 8
    nc, in_maps, plan = _prepare(
        feats, W, gamma, beta, in_map, out_map, n_out, n_cores)
    res = run_bass_kernel_spmd(nc, in_maps, list(range(n_cores)))
    rows = plan["rows_per_core"]
    out = np.concatenate(
        [np.asarray(res.results[c]["y"]).reshape(-1, 64)[:rows]
         for c in range(n_cores)], axis=0)
    return out.astype(np.float32)


# revision 30
# speedup vs baseline: 2.4013x; 1.2180x over previous
"""Trainium2 Bass kernel for nn_BasicDeconvolutionBlock.

Reference computation:
    gathered = feats[in_map]                         # [K, M, Cin]
    contrib  = einsum('kmc,kcd->kmd', gathered, W)   # [K, M, Cout]
    out      = zeros([n_out, Cout]).at[out_map].add(contrib)
    y        = relu(batchnorm(out))                  # batch stats over n_out rows

Strategy (8 NeuronCores, SPMD):
  - Host routes each (k, m) pair to the core owning its output row
    (row blocks of n_out/8).  Per core ~169k pairs.
  - Slot stream per core: for each feats chunk (int16 gather range, 32768
    rows), pairs are laid out in (stage, k) groups with NO per-group
    padding (shared caps = max pair count over cores; ragged matmul
    pieces handle group boundaries).  The stream is cut into 896-slot
    windows (the SWDGE per-call index cap; 1024+ wedges the Q7 ucode).
    Each window is one dma_gather call AND one dma_scatter_add call.
  - Duplicate out-rows inside one scatter call race in hardware, so the
    host EVICTS same-(window,row) duplicates to overflow stages appended
    to the same chunk run (iterated until dup-free).  Cross-window dups
    are safe: same-bank windows serialize via Tile WAW deps, and the two
    HBM accumulator banks alternate by window parity so adjacent windows
    overlap.
  - Gather: feats pre-cast to bf16, padded to 128 channels (256B rows);
    dma_gather(transpose=True) yields channel-major G[128, slots].
  - GEMM: per window, ragged pieces (cut at k-group and 128-tile
    boundaries) matmul into ONE PSUM bank region [128, 448] f32
    (slot-major: partition=slot%128, col block=slot//128), then a single
    DVE copy converts to an fp16 slab.
  - Scatter: dma_scatter_add (CCE add, fp16, elem 64, row stride 256B)
    into acc banks [acc_rows+128, 128] fp16 (cols 64:128 unused pad to
    satisfy the 256B row-stride requirement); pad/hole/evicted slots go
    to a dump row beyond acc_rows.
  - BN: fold banks, ones-matmul row sums + sum of squares, [2,64]
    AllReduce across 8 cores, normalize + ReLU, write [25088, 64] f32.
"""

import numpy as np

import sys

sys.path.insert(0, "/opt/trn_rl_repo")

import ml_dtypes  # noqa: E402

from concourse import bacc, bass, mybir  # noqa: E402
import concourse.tile as tile  # noqa: E402

BN_EPS = 1e-5
CHUNK = 32768  # int16 gather index range per feats chunk
SEG = 896  # max indices per SWDGE call (1024+ wedges the Q7 ucode)
NBANKS = 2
F32 = mybir.dt.float32
FP16 = mybir.dt.float16
BF16 = mybir.dt.bfloat16
I16 = mybir.dt.int16


def _roundup(x, m):
    return (x + m - 1) // m * m


def _occ_rank(keys):
    """Rank of each element among equal values of `keys` (stable)."""
    order = np.argsort(keys, kind="stable")
    ks = keys[order]
    n = len(ks)
    first = np.ones(n, dtype=bool)
    if n:
        first[1:] = ks[1:] != ks[:-1]
    grp = np.maximum.accumulate(np.where(first, np.arange(n), 0))
    rank_sorted = np.arange(n) - grp
    out = np.empty(n, dtype=np.int64)
    out[order] = rank_sorted
    return out


def _route(in_map, out_map, n_out, n_cores):
    """Host-side routing.  Returns (plan, gidx_all, sidx_all).

    plan.windows: list of dicts (c, slot0, n, gcol0, pieces=[(k, off, len)])
    where slot0/gcol0 are global and off is window-relative.
    """
    K, M = in_map.shape
    rows_per_core = n_out // n_cores
    assert rows_per_core * n_cores == n_out
    acc_rows = _roundup(rows_per_core, 128)
    dump_row = acc_rows
    nchunk = _roundup(int(in_map.max()) + 1, CHUNK) // CHUNK

    k_idx = np.repeat(np.arange(K, dtype=np.int64), M)
    in_flat = in_map.ravel().astype(np.int64)
    out_flat = out_map.ravel().astype(np.int64)
    core_of = out_flat // rows_per_core
    row_local = out_flat % rows_per_core
    chunk_of = in_flat // CHUNK
    idx_local = in_flat % CHUNK

    # pending pairs per (core, chunk)
    pend = {}
    for ci in range(n_cores):
        sel = np.nonzero(core_of == ci)[0]
        for c in range(nchunk):
            s2 = sel[chunk_of[sel] == c]
            pend[(ci, c)] = (k_idx[s2], idx_local[s2], row_local[s2])

    # iterate stages per chunk until dup-free; build shared group layout
    run_kk = [[] for _ in range(nchunk)]  # per-chunk per-slot k (shared)
    run_len = np.zeros(nchunk, dtype=np.int64)
    # per (core, chunk): placed slot -> (gidx value, sidx value)
    placed = {key: ([], [], []) for key in pend}  # slots, gvals, svals

    for c in range(nchunk):
        stage = 0
        used = {ci: np.empty(0, dtype=np.int64) for ci in range(n_cores)}
        while True:
            counts = np.zeros((n_cores, K), dtype=np.int64)
            for ci in range(n_cores):
                kk, _, _ = pend[(ci, c)]
                if len(kk):
                    np.add.at(counts[ci], kk, 1)
            caps = counts.max(axis=0)
            # 64-align group sizes so matmul piece starts land on PSUM base
            # partitions 0/64 (PE tile_position + AP base_partition limits)
            caps = (caps + 63) // 64 * 64
            if caps.sum() == 0:
                break
            assert stage < 40, "eviction did not converge"
            if stage >= 2:
                # late stages start on a fresh window so high-multiplicity
                # rows always find a free (window,row) slot -> convergence
                tgt = _roundup(int(run_len[c]), SEG)
                if tgt > run_len[c]:
                    lastk = run_kk[c][-1] if run_kk[c] else 0
                    run_kk[c].extend([lastk] * int(tgt - run_len[c]))
                    run_len[c] = tgt
            g0 = np.zeros(K + 1, dtype=np.int64)
            g0[1:] = np.cumsum(caps)
            base = run_len[c]
            for k in range(K):
                run_kk[c].extend([k] * int(caps[k]))
            run_len[c] += caps.sum()
            for ci in range(n_cores):
                kk, iv, rv = pend[(ci, c)]
                if not len(kk):
                    pend[(ci, c)] = (kk, iv, rv)
                    continue
                # in-group order: (occ among same (k,row), row) to spread
                # a row's duplicates across the group's windows
                occ = _occ_rank(kk * rows_per_core + rv)
                order = np.lexsort((rv, occ, kk))
                kk, iv, rv = kk[order], iv[order], rv[order]
                starts = np.concatenate(
                    [[0], np.cumsum(np.bincount(kk, minlength=K))])
                rank = np.arange(len(kk)) - starts[kk]
                slot = base + g0[kk] + rank
                win = slot // SEG
                key = win * rows_per_core + rv
                # keep the first pair (in provisional order) per (win,row),
                # excluding (win,row) pairs already used by earlier stages
                fresh = ~np.isin(key, used[ci])
                order2 = np.lexsort((np.arange(len(key)), key))
                ks = key[order2]
                first = np.ones(len(ks), dtype=bool)
                if len(ks):
                    first[1:] = ks[1:] != ks[:-1]
                keep = np.zeros(len(key), dtype=bool)
                keep[order2[first]] = True
                keep &= fresh
                used[ci] = np.concatenate([used[ci], key[keep]])
                ps, pg, pv = placed[(ci, c)]
                ps.append(slot[keep])
                pg.append(iv[keep])
                pv.append(rv[keep])
                pend[(ci, c)] = (kk[~keep], iv[~keep], rv[~keep])
            stage += 1

    # pad runs to 128; build windows and global offsets
    run_pad = np.array([_roundup(int(x), 128) for x in run_len], dtype=np.int64)
    for c in range(nchunk):
        lastk = run_kk[c][-1] if run_kk[c] else 0
        run_kk[c].extend([lastk] * int(run_pad[c] - run_len[c]))
    run0 = np.zeros(nchunk + 1, dtype=np.int64)
    run0[1:] = np.cumsum(run_pad)
    total_slots = int(run0[-1])

    windows = []
    for c in range(nchunk):
        kkarr = np.array(run_kk[c], dtype=np.int64)
        off = 0
        while off < run_pad[c]:
            n = int(min(SEG, run_pad[c] - off))
            # pieces: boundaries at k-change and 128-tile cuts, then
            # subdivided to satisfy PE tile-position rules (base 0: len
            # <=128, base 64: len <=64, base 32/96: len <=32)
            pieces = []
            j = 0
            while j < n:
                k = int(kkarr[off + j])
                e = j + 1
                while e < n and kkarr[off + e] == k and e % 128 != 0:
                    e += 1
                p0, rem = j, e - j
                while rem > 0:
                    bp = p0 % 128
                    assert bp in (0, 64), (bp, j, e)
                    allowed = 128 if bp == 0 else 64
                    take = min(rem, allowed)
                    pieces.append((k, p0, take))
                    p0 += take
                    rem -= take
                j = e
            windows.append(
                dict(
                    c=c,
                    slot0=int(run0[c] + off),
                    n=n,
                    gcol0=int((run0[c] + off) // 16),
                    pieces=pieces,
                )
            )
            off += n

    # per-core packed idx arrays (16-wrapped, replicated to 128 partitions);
    # gather and scatter indices for each window are interleaved into ONE
    # tensor so a single DMA per window loads both
    gcols = total_slots // 16
    gsidx_all = np.zeros((n_cores, 128, 2 * gcols), dtype=np.int16)
    for ci in range(n_cores):
        gvals = np.zeros(total_slots, dtype=np.int16)
        svals = np.full(total_slots, dump_row, dtype=np.int16)
        for c in range(nchunk):
            ps, pg, pv = placed[(ci, c)]
            if not ps:
                continue
            slots = np.concatenate(ps) + run0[c]
            gvals[slots] = np.concatenate(pg).astype(np.int16)
            svals[slots] = np.concatenate(pv).astype(np.int16)
        gwrap = np.tile(gvals.reshape(-1, 16).T, (8, 1))
        swrap = np.tile(svals.reshape(-1, 16).T, (8, 1))
        for w in windows:
            c0, nw = w["gcol0"], w["n"] // 16
            gsidx_all[ci][:, 2 * c0:2 * c0 + nw] = gwrap[:, c0:c0 + nw]
            gsidx_all[ci][:, 2 * c0 + nw:2 * c0 + 2 * nw] = (
                swrap[:, c0:c0 + nw])

    plan = dict(
        nchunk=nchunk,
        rows_per_core=rows_per_core,
        acc_rows=acc_rows,
        acc_total=acc_rows + 128,
        dump_row=dump_row,
        windows=windows,
        total_slots=total_slots,
        gcols=gcols,
    )
    return plan, gsidx_all


def _build(plan, n_out, ftab_rows, n_cores):
    """Trace the Bass program.  Returns nc."""
    nc = bacc.Bacc("TRN2", target_bir_lowering=False, debug=False)

    K = 27
    Cout = 64
    acc_rows, acc_total = plan["acc_rows"], plan["acc_total"]
    windows = plan["windows"]
    Tb = acc_rows // 128  # BN column tiles

    ftab = nc.dram_tensor("ftab", [ftab_rows, 128], BF16, kind="ExternalInput")
    wt = nc.dram_tensor("wt", [128, K * Cout], BF16, kind="ExternalInput")
    gsidx = nc.dram_tensor("gsidx", [128, 2 * plan["gcols"]], I16,
                           kind="ExternalInput")
    gb = nc.dram_tensor("gb", [2, Cout], F32, kind="ExternalInput")
    accs = [
        nc.dram_tensor(f"acc{b}", [acc_total, 128], FP16) for b in range(NBANKS)
    ]
    cc_in = nc.dram_tensor("cc_in", [2, Cout], F32)
    cc_out = nc.dram_tensor("cc_out", [2, Cout], F32, addr_space="Shared")
    # 3D view of the [acc_rows, 64] output (row = p*Tb + t) so row-half
    # writes can overlap the normalize pipeline
    y = nc.dram_tensor("y", [128, acc_rows // 128, Cout], F32,
                       kind="ExternalOutput")

    with tile.TileContext(nc) as tc:
        with (
            tc.tile_pool(name="const", bufs=1) as cpool,
            tc.tile_pool(name="gpool", bufs=6) as gpool,
            tc.tile_pool(name="slab", bufs=6) as slabpool,
            tc.tile_pool(name="gixp", bufs=6) as gixpool,
            tc.tile_pool(name="psum", bufs=7, space="PSUM") as pspool,
        ):
            w_sb = cpool.tile([128, K * Cout], BF16, tag="w")
            nc.sync.dma_start(out=w_sb[:, :], in_=wt[:, :])
            zed = cpool.tile([128, 6400], FP16, tag="zed")
            nc.vector.memset(zed[:, :], 0.0)
            zrows = 128 * 6400 // 128  # 6400 rows per DMA
            for bank in accs:
                r0 = 0
                while r0 < acc_total:
                    rcnt = min(zrows, acc_total - r0)
                    nc.sync.dma_start(
                        out=bank[r0:r0 + rcnt, :],
                        in_=zed[:, :rcnt],
                    )
                    r0 += rcnt

            # software pipelining: emit gathers LOOKAHEAD windows ahead of
            # their scatters so the in-order Pool queue never head-of-line
            # blocks on a scatter whose slab is still being produced
            LOOKAHEAD = 3
            pend_scat = []  # (wi, gi, slab, ntile, nw)

            def emit_scatter(ent):
                wi, gi, slab, ntile, nw, n = ent
                nc.gpsimd.dma_scatter_add(
                    out_ap=accs[wi % NBANKS][:, 0:64],
                    in_ap=slab[:, :ntile, :],
                    idxs_ap=gi[:, nw:2 * nw],
                    num_idxs=n,
                    num_idxs_reg=n,
                    elem_size=64,
                    elem_step=128,
                )

            for wi, w in enumerate(windows):
                n = w["n"]
                ntile = n // 128
                nw = n // 16
                gi = gixpool.tile([128, 2 * (SEG // 16)], I16, tag="gi")
                nc.sync.dma_start(
                    out=gi[:, :2 * nw],
                    in_=gsidx[:, 2 * w["gcol0"]:2 * w["gcol0"] + 2 * nw],
                )
                g = gpool.tile([128, 1, SEG], BF16, tag="g")
                c = w["c"]
                nc.gpsimd.dma_gather(
                    out_ap=g[:, :, :n],
                    in_ap=ftab[c * CHUNK:min((c + 1) * CHUNK, ftab_rows), :],
                    idxs_ap=gi[:, :nw],
                    num_idxs=n,
                    num_idxs_reg=n,
                    elem_size=128,
                    transpose=True,
                )
                ps = pspool.tile([128, 8, 64], F32, tag="ps")
                for (k, off, ln) in w["pieces"]:
                    p0 = off % 128
                    t = off // 128
                    nc.tensor.matmul(
                        out=ps[p0:p0 + ln, t, :],
                        lhsT=g[:, 0, off:off + ln],
                        rhs=w_sb[:, k * Cout:(k + 1) * Cout],
                        start=True, stop=True,
                    )
                slab = slabpool.tile([128, SEG // 128, 64], FP16, tag="slab")
                nc.vector.tensor_copy(
                    out=slab[:, :ntile, :], in_=ps[:, :ntile, :])
                pend_scat.append((wi, gi, slab, ntile, nw, n))
                if len(pend_scat) > LOOKAHEAD:
                    emit_scatter(pend_scat.pop(0))
            for ent in pend_scat:
                emit_scatter(ent)

        # ---- BN phase ----
        Tp = _roundup(Tb, 8)  # stat tiles padded so all matmuls are 512-wide
        F32R = mybir.dt.float32r
        with (
            tc.tile_pool(name="bn", bufs=1) as bnpool,
            tc.tile_pool(name="bns", bufs=4) as bnspool,
            tc.tile_pool(name="bnp", bufs=2, space="PSUM") as bnps,
        ):
            out_sb = bnpool.tile([128, Tp, 64], F32, tag="outsb")
            gam_t = bnspool.tile([1, 64], F32, tag="gam")
            bet_t = bnspool.tile([1, 64], F32, tag="bet")
            nc.sync.dma_start(out=gam_t[:, :], in_=gb[0:1, :])
            nc.sync.dma_start(out=bet_t[:, :], in_=gb[1:2, :])
            with tc.tile_pool(name="bnb", bufs=2) as bnbpool:
                b0 = bnbpool.tile([128, Tb, 128], FP16, tag="bank")
                nc.sync.dma_start(out=b0[:, :, :], in_=accs[0][0:acc_rows, :])
                b1 = bnbpool.tile([128, Tb, 128], FP16, tag="bank")
                nc.sync.dma_start(out=b1[:, :, :], in_=accs[1][0:acc_rows, :])
                if Tp > Tb:
                    nc.vector.memset(out_sb[:, Tb:Tp, :], 0.0)
                nc.vector.tensor_tensor(
                    out=out_sb[:, 0:Tb, :], in0=b0[:, :, 0:64],
                    in1=b1[:, :, 0:64], op=mybir.AluOpType.add)
            ones = bnpool.tile([128, 1], BF16, tag="ones")
            nc.vector.memset(ones[:, :], 1.0)
            sum_ps = bnps.tile([1, 512], F32, tag="sum")
            sq_ps = bnps.tile([1, 512], F32, tag="sq")
            ngrp = Tp // 8
            for i in range(ngrp):
                xbt = bnspool.tile([128, 8, 64], BF16, tag="xbt")
                nc.vector.tensor_copy(
                    out=xbt[:, :, :], in_=out_sb[:, 8 * i:8 * i + 8, :])
                nc.tensor.matmul(
                    out=sum_ps[:, :], lhsT=ones[:, :], rhs=xbt[:, :, :],
                    start=(i == 0), stop=(i == ngrp - 1),
                    skip_group_check=True,
                )
                sqt = bnspool.tile([128, 8, 64], BF16, tag="sqt")
                nc.vector.tensor_tensor(
                    out=sqt[:, :, :], in0=xbt[:, :, :], in1=xbt[:, :, :],
                    op=mybir.AluOpType.mult)
                nc.tensor.matmul(
                    out=sq_ps[:, :], lhsT=ones[:, :], rhs=sqt[:, :, :],
                    start=(i == 0), stop=(i == ngrp - 1),
                    skip_group_check=True,
                )
            st0 = bnspool.tile([1, 512], F32, tag="st0")
            st1 = bnspool.tile([1, 512], F32, tag="st1")
            nc.vector.tensor_copy(out=st0[:, :], in_=sum_ps[:, :])
            nc.vector.tensor_copy(out=st1[:, :], in_=sq_ps[:, :])
            for st in (st0, st1):
                nc.vector.tensor_tensor(
                    out=st[:, 0:256], in0=st[:, 0:256], in1=st[:, 256:512],
                    op=mybir.AluOpType.add)
                nc.vector.tensor_tensor(
                    out=st[:, 0:128], in0=st[:, 0:128], in1=st[:, 128:256],
                    op=mybir.AluOpType.add)
                nc.vector.tensor_tensor(
                    out=st[:, 0:64], in0=st[:, 0:64], in1=st[:, 64:128],
                    op=mybir.AluOpType.add)
            nc.sync.dma_start(out=cc_in[0:1, :], in_=st0[:, 0:64])
            nc.sync.dma_start(out=cc_in[1:2, :], in_=st1[:, 0:64])
            nc.gpsimd.collective_compute(
                "AllReduce",
                mybir.AluOpType.add,
                ins=[cc_in[:, :]],
                outs=[cc_out[:, :]],
                replica_groups=[list(range(n_cores))],
            )
            gs0 = bnspool.tile([1, 64], F32, tag="gs0")
            gs1 = bnspool.tile([1, 64], F32, tag="gs1")
            nc.sync.dma_start(out=gs0[:, :], in_=cc_out[0:1, :])
            nc.sync.dma_start(out=gs1[:, :], in_=cc_out[1:2, :])

            inv_n = 1.0 / float(n_out)
            mean_t = bnspool.tile([1, 64], F32, tag="mean")
            ex2_t = bnspool.tile([1, 64], F32, tag="ex2")
            var_t = bnspool.tile([1, 64], F32, tag="var")
            sd_t = bnspool.tile([1, 64], F32, tag="sd")
            rs_t = bnspool.tile([1, 64], F32, tag="rs")
            a_t = bnspool.tile([1, 64], F32, tag="a")
            b_t = bnspool.tile([1, 64], F32, tag="b")
            nc.vector.tensor_scalar_mul(mean_t[:, :], gs0[:, :], inv_n)
            nc.vector.tensor_scalar_mul(ex2_t[:, :], gs1[:, :], inv_n)
            nc.vector.tensor_tensor(
                out=var_t[:, :], in0=mean_t[:, :], in1=mean_t[:, :],
                op=mybir.AluOpType.mult)
            nc.vector.tensor_tensor(
                out=var_t[:, :], in0=ex2_t[:, :], in1=var_t[:, :],
                op=mybir.AluOpType.subtract)
            nc.vector.tensor_scalar_add(var_t[:, :], var_t[:, :], BN_EPS)
            nc.scalar.activation(
                out=sd_t[:, :], in_=var_t[:, :],
                func=mybir.ActivationFunctionType.Sqrt)
            nc.vector.reciprocal(out=rs_t[:, :], in_=sd_t[:, :])
            nc.vector.tensor_tensor(
                out=a_t[:, :], in0=gam_t[:, :], in1=rs_t[:, :],
                op=mybir.AluOpType.mult)
            nc.vector.tensor_tensor(
                out=b_t[:, :], in0=mean_t[:, :], in1=a_t[:, :],
                op=mybir.AluOpType.mult)
            nc.vector.tensor_tensor(
                out=b_t[:, :], in0=bet_t[:, :], in1=b_t[:, :],
                op=mybir.AluOpType.subtract)
            # broadcast [1,64] -> [128,64] via PE (ones[128,1] @ row)
            ones_row = bnspool.tile([1, 128], F32, tag="ones_row")
            nc.vector.memset(ones_row[:, :], 1.0)
            a_full = bnspool.tile([128, 64], F32, tag="afull")
            b_full = bnspool.tile([128, 64], F32, tag="bfull")
            ab_ps = bnps.tile([128, 64], F32, tag="abps")
            nc.tensor.matmul(
                out=ab_ps[:, :], lhsT=ones_row[:, :], rhs=a_t[:, :],
                start=True, stop=True)
            nc.vector.tensor_copy(out=a_full[:, :], in_=ab_ps[:, :])
            nc.tensor.matmul(
                out=ab_ps[:, :], lhsT=ones_row[:, :], rhs=b_t[:, :],
                start=True, stop=True)
            nc.vector.tensor_copy(out=b_full[:, :], in_=ab_ps[:, :])
            # fused normalize + relu over row halves, each half's y write
            # overlapping the next half's compute
            half = Tb // 2
            for (t0, t1) in ((0, half), (half, Tb)):
                nt = t1 - t0
                nc.vector.tensor_tensor(
                    out=out_sb[:, t0:t1, :], in0=out_sb[:, t0:t1, :],
                    in1=a_full[:, None, :].to_broadcast((128, nt, 64)),
                    op=mybir.AluOpType.mult)
                nc.vector.tensor_tensor(
                    out=out_sb[:, t0:t1, :], in0=out_sb[:, t0:t1, :],
                    in1=b_full[:, None, :].to_broadcast((128, nt, 64)),
                    op=mybir.AluOpType.add)
                nc.scalar.activation(
                    out=out_sb[:, t0:t1, :], in_=out_sb[:, t0:t1, :],
                    func=mybir.ActivationFunctionType.Relu)
                nc.sync.dma_start(
                    out=y[:, t0:t1, :], in_=out_sb[:, t0:t1, :])

    nc.compile()
    return nc


def _prepare(feats, W, gamma, beta, in_map, out_map, n_out, n_cores):
    """Host prep shared by kernel() and tests.  Returns (nc, in_maps, plan)."""
    n_out = int(n_out)
    K, Cin, Cout = W.shape
    assert Cin == 64 and Cout == 64
    in_map = np.asarray(in_map, dtype=np.int64)
    out_map = np.asarray(out_map, dtype=np.int64)
    feats = np.asarray(feats, dtype=np.float32)
    W = np.asarray(W, dtype=np.float32)

    plan, gsidx_all = _route(in_map, out_map, n_out, n_cores)

    ftab_rows = _roundup(feats.shape[0], CHUNK)
    ftab = np.zeros((ftab_rows, 128), dtype=ml_dtypes.bfloat16)
    ftab[:feats.shape[0], :64] = feats.astype(ml_dtypes.bfloat16)

    wt = np.zeros((128, K * 64), dtype=ml_dtypes.bfloat16)
    wt[:64, :] = (
        W.transpose(1, 0, 2).reshape(64, K * 64).astype(ml_dtypes.bfloat16))

    gb = np.stack([np.asarray(gamma, np.float32),
                   np.asarray(beta, np.float32)])

    nc = _build(plan, n_out, ftab_rows, n_cores)
    in_maps = [
        dict(ftab=ftab, wt=wt, gsidx=gsidx_all[c], gb=gb)
        for c in range(n_cores)
    ]
    return nc, in_maps, plan


def kernel(feats, W, gamma, beta, in_map, out_map, n_out):
    from concourse.bass_utils import run_bass_kernel_spmd

    n_cores = 8
    nc, in_maps, plan = _prepare(
        feats, W, gamma, beta, in_map, out_map, n_out, n_cores)
    res = run_bass_kernel_spmd(nc, in_maps, list(range(n_cores)))
    rows = plan["rows_per_core"]
    out = np.concatenate(
        [np.asarray(res.results[c]["y"]).reshape(-1, 64)[:rows]
         for c in range(n_cores)], axis=0)
    return out.astype(np.float32)


# revision 40
# speedup vs baseline: 2.8017x; 1.1667x over previous
"""Trainium2 Bass kernel for nn_BasicDeconvolutionBlock.

Reference computation:
    gathered = feats[in_map]                         # [K, M, Cin]
    contrib  = einsum('kmc,kcd->kmd', gathered, W)   # [K, M, Cout]
    out      = zeros([n_out, Cout]).at[out_map].add(contrib)
    y        = relu(batchnorm(out))                  # batch stats over n_out rows

Strategy (8 NeuronCores, SPMD):
  - Host routes each (k, m) pair to the core owning its output row
    (row blocks of n_out/8).  Per core ~169k pairs.
  - Slot stream per core: for each feats chunk (int16 gather range, 32768
    rows), pairs are laid out in (stage, k) groups with NO per-group
    padding (shared caps = max pair count over cores; ragged matmul
    pieces handle group boundaries).  The stream is cut into 896-slot
    windows (the SWDGE per-call index cap; 1024+ wedges the Q7 ucode).
    Each window is one dma_gather call AND one dma_scatter_add call.
  - Duplicate out-rows inside one scatter call race in hardware, so the
    host EVICTS same-(window,row) duplicates to overflow stages appended
    to the same chunk run (iterated until dup-free).  Cross-window dups
    are safe: same-bank windows serialize via Tile WAW deps, and the two
    HBM accumulator banks alternate by window parity so adjacent windows
    overlap.
  - Gather: feats pre-cast to bf16, padded to 128 channels (256B rows);
    dma_gather(transpose=True) yields channel-major G[128, slots].
  - GEMM: per window, ragged pieces (cut at k-group and 128-tile
    boundaries) matmul into ONE PSUM bank region [128, 448] f32
    (slot-major: partition=slot%128, col block=slot//128), then a single
    DVE copy converts to an fp16 slab.
  - Scatter: dma_scatter_add (CCE add, fp16, elem 64, row stride 256B)
    into acc banks [acc_rows+128, 128] fp16 (cols 64:128 unused pad to
    satisfy the 256B row-stride requirement); pad/hole/evicted slots go
    to a dump row beyond acc_rows.
  - BN: fold banks, ones-matmul row sums + sum of squares, [2,64]
    AllReduce across 8 cores, normalize + ReLU, write [25088, 64] f32.
"""

import numpy as np

import sys

sys.path.insert(0, "/opt/trn_rl_repo")

import ml_dtypes  # noqa: E402

from concourse import bacc, bass, mybir  # noqa: E402
import concourse.tile as tile  # noqa: E402

import os

BN_EPS = 1e-5
CHUNK = 32768  # int16 gather index range per feats chunk
SEG = int(os.environ.get("DECONV_SEG", "896"))  # max indices per SWDGE call
SINGLE_PACKET = os.environ.get("DECONV_SP", "1") == "1"
NBANKS = 2
F32 = mybir.dt.float32
FP16 = mybir.dt.float16
BF16 = mybir.dt.bfloat16
I16 = mybir.dt.int16


def _roundup(x, m):
    return (x + m - 1) // m * m


def _occ_rank(keys):
    """Rank of each element among equal values of `keys` (stable)."""
    order = np.argsort(keys, kind="stable")
    ks = keys[order]
    n = len(ks)
    first = np.ones(n, dtype=bool)
    if n:
        first[1:] = ks[1:] != ks[:-1]
    grp = np.maximum.accumulate(np.where(first, np.arange(n), 0))
    rank_sorted = np.arange(n) - grp
    out = np.empty(n, dtype=np.int64)
    out[order] = rank_sorted
    return out


def _route(in_map, out_map, n_out, n_cores):
    """Host-side routing.  Returns (plan, gidx_all, sidx_all).

    plan.windows: list of dicts (c, slot0, n, gcol0, pieces=[(k, off, len)])
    where slot0/gcol0 are global and off is window-relative.
    """
    K, M = in_map.shape
    rows_per_core = n_out // n_cores
    assert rows_per_core * n_cores == n_out
    acc_rows = _roundup(rows_per_core, 128)
    dump_row = acc_rows
    nchunk = _roundup(int(in_map.max()) + 1, CHUNK) // CHUNK

    k_idx = np.repeat(np.arange(K, dtype=np.int64), M)
    in_flat = in_map.ravel().astype(np.int64)
    out_flat = out_map.ravel().astype(np.int64)
    core_of = out_flat // rows_per_core
    row_local = out_flat % rows_per_core
    chunk_of = in_flat // CHUNK
    idx_local = in_flat % CHUNK

    # pending pairs per (core, chunk)
    pend = {}
    for ci in range(n_cores):
        sel = np.nonzero(core_of == ci)[0]
        for c in range(nchunk):
            s2 = sel[chunk_of[sel] == c]
            pend[(ci, c)] = (k_idx[s2], idx_local[s2], row_local[s2])

    # iterate stages per chunk until dup-free; build shared group layout
    run_kk = [[] for _ in range(nchunk)]  # per-chunk per-slot k (shared)
    run_len = np.zeros(nchunk, dtype=np.int64)
    # per (core, chunk): placed slot -> (gidx value, sidx value)
    placed = {key: ([], [], []) for key in pend}  # slots, gvals, svals

    for c in range(nchunk):
        stage = 0
        used = {ci: np.empty(0, dtype=np.int64) for ci in range(n_cores)}
        while True:
            counts = np.zeros((n_cores, K), dtype=np.int64)
            for ci in range(n_cores):
                kk, _, _ = pend[(ci, c)]
                if len(kk):
                    np.add.at(counts[ci], kk, 1)
            caps = counts.max(axis=0)
            # 32-align group sizes; matmul pieces handle base partitions
            # 0/32/64, and a 96-start run is covered by a wide base-64
            # matmul emitted first then overwritten (see piece builder)
            caps = (caps + 31) // 32 * 32
            if caps.sum() == 0:
                break
            assert stage < 40, "eviction did not converge"
            if stage >= 2:
                # late stages start on a fresh window so high-multiplicity
                # rows always find a free (window,row) slot -> convergence
                tgt = _roundup(int(run_len[c]), SEG)
                if tgt > run_len[c]:
                    lastk = run_kk[c][-1] if run_kk[c] else 0
                    run_kk[c].extend([lastk] * int(tgt - run_len[c]))
                    run_len[c] = tgt
            g0 = np.zeros(K + 1, dtype=np.int64)
            g0[1:] = np.cumsum(caps)
            base = run_len[c]
            for k in range(K):
                run_kk[c].extend([k] * int(caps[k]))
            run_len[c] += caps.sum()
            for ci in range(n_cores):
                kk, iv, rv = pend[(ci, c)]
                if not len(kk):
                    pend[(ci, c)] = (kk, iv, rv)
                    continue
                # in-group order: (occ among same (k,row), row) to spread
                # a row's duplicates across the group's windows
                occ = _occ_rank(kk * rows_per_core + rv)
                order = np.lexsort((rv, occ, kk))
                kk, iv, rv = kk[order], iv[order], rv[order]
                starts = np.concatenate(
                    [[0], np.cumsum(np.bincount(kk, minlength=K))])
                rank = np.arange(len(kk)) - starts[kk]
                slot = base + g0[kk] + rank
                win = slot // SEG
                key = win * rows_per_core + rv
                # keep the first pair (in provisional order) per (win,row),
                # excluding (win,row) pairs already used by earlier stages
                fresh = ~np.isin(key, used[ci])
                order2 = np.lexsort((np.arange(len(key)), key))
                ks = key[order2]
                first = np.ones(len(ks), dtype=bool)
                if len(ks):
                    first[1:] = ks[1:] != ks[:-1]
                keep = np.zeros(len(key), dtype=bool)
                keep[order2[first]] = True
                keep &= fresh
                used[ci] = np.concatenate([used[ci], key[keep]])
                ps, pg, pv = placed[(ci, c)]
                ps.append(slot[keep])
                pg.append(iv[keep])
                pv.append(rv[keep])
                pend[(ci, c)] = (kk[~keep], iv[~keep], rv[~keep])
            stage += 1

    # pad runs to 128; build windows and global offsets
    run_pad = np.array([_roundup(int(x), 128) for x in run_len], dtype=np.int64)
    for c in range(nchunk):
        lastk = run_kk[c][-1] if run_kk[c] else 0
        run_kk[c].extend([lastk] * int(run_pad[c] - run_len[c]))
    run0 = np.zeros(nchunk + 1, dtype=np.int64)
    run0[1:] = np.cumsum(run_pad)
    total_slots = int(run0[-1])

    windows = []
    for c in range(nchunk):
        kkarr = np.array(run_kk[c], dtype=np.int64)
        off = 0
        while off < run_pad[c]:
            n = int(min(SEG, run_pad[c] - off))
            # pieces: k-runs cut at 128-tile boundaries, subdivided for the
            # PE tile-position rules (legal PSUM bases 0/32/64; base 0 len
            # <=128, base 32 len <=32, base 64 len <=64).  A run starting
            # at 96 has no legal base: emit a WIDE base-64 matmul for it
            # first, then the normal pieces covering [64,96) overwrite the
            # wrong-k half (start=True resets the written region).
            tile_runs = {}
            j = 0
            while j < n:
                k = int(kkarr[off + j])
                e = j + 1
                while e < n and kkarr[off + e] == k:
                    e += 1
                s = j
                while s < e:
                    t = s // 128
                    te = min(e, (t + 1) * 128)
                    tile_runs.setdefault(t, []).append((k, s, te))
                    s = te
                j = e
            pieces = []
            for t in sorted(tile_runs):
                runs = tile_runs[t]
                for (k, s, e) in runs:
                    if s % 128 == 96:
                        pieces.append((k, t * 128 + 64, e - (t * 128 + 64)))
                for (k, s, e) in runs:
                    if s % 128 == 96:
                        continue
                    p0, rem = s, e - s
                    while rem > 0:
                        bp = p0 % 128
                        allowed = 128 if bp == 0 else (64 if bp == 64 else 32)
                        take = min(rem, allowed)
                        pieces.append((k, p0, take))
                        p0 += take
                        rem -= take
            windows.append(
                dict(
                    c=c,
                    slot0=int(run0[c] + off),
                    n=n,
                    gcol0=int((run0[c] + off) // 16),
                    pieces=pieces,
                )
            )
            off += n

    # per-core packed idx arrays (16-wrapped, replicated to 128 partitions);
    # gather and scatter indices for each window are interleaved into ONE
    # tensor so a single DMA per window loads both
    gcols = total_slots // 16
    gsidx_all = np.zeros((n_cores, 128, 2 * gcols), dtype=np.int16)
    for ci in range(n_cores):
        gvals = np.zeros(total_slots, dtype=np.int16)
        svals = np.full(total_slots, dump_row, dtype=np.int16)
        for c in range(nchunk):
            ps, pg, pv = placed[(ci, c)]
            if not ps:
                continue
            slots = np.concatenate(ps) + run0[c]
            gvals[slots] = np.concatenate(pg).astype(np.int16)
            svals[slots] = np.concatenate(pv).astype(np.int16)
        gwrap = np.tile(gvals.reshape(-1, 16).T, (8, 1))
        swrap = np.tile(svals.reshape(-1, 16).T, (8, 1))
        for w in windows:
            c0, nw = w["gcol0"], w["n"] // 16
            gsidx_all[ci][:, 2 * c0:2 * c0 + nw] = gwrap[:, c0:c0 + nw]
            gsidx_all[ci][:, 2 * c0 + nw:2 * c0 + 2 * nw] = (
                swrap[:, c0:c0 + nw])

    plan = dict(
        nchunk=nchunk,
        rows_per_core=rows_per_core,
        acc_rows=acc_rows,
        acc_total=acc_rows + 128,
        dump_row=dump_row,
        windows=windows,
        total_slots=total_slots,
        gcols=gcols,
    )
    return plan, gsidx_all


def _build(plan, n_out, ftab_rows, n_cores):
    """Trace the Bass program.  Returns nc."""
    nc = bacc.Bacc("TRN2", target_bir_lowering=False, debug=False)

    K = 27
    Cout = 64
    acc_rows, acc_total = plan["acc_rows"], plan["acc_total"]
    windows = plan["windows"]
    Tb = acc_rows // 128  # BN column tiles

    ftab = nc.dram_tensor("ftab", [ftab_rows, 128], BF16, kind="ExternalInput")
    wt = nc.dram_tensor("wt", [128, K * Cout], BF16, kind="ExternalInput")
    gsidx = nc.dram_tensor("gsidx", [128, 2 * plan["gcols"]], I16,
                           kind="ExternalInput")
    gb = nc.dram_tensor("gb", [2, Cout], F32, kind="ExternalInput")
    accs = [
        nc.dram_tensor(f"acc{b}", [acc_total, 128], FP16) for b in range(NBANKS)
    ]
    cc_in = nc.dram_tensor("cc_in", [2, Cout], F32)
    cc_out = nc.dram_tensor("cc_out", [2, Cout], F32, addr_space="Shared")
    # 3D view of the [acc_rows, 64] output (row = p*Tb + t) so row-half
    # writes can overlap the normalize pipeline
    y = nc.dram_tensor("y", [128, acc_rows // 128, Cout], F32,
                       kind="ExternalOutput")

    with tile.TileContext(nc) as tc:
        with (
            tc.tile_pool(name="const", bufs=1) as cpool,
            tc.tile_pool(name="gpool", bufs=8) as gpool,
            tc.tile_pool(name="slab", bufs=8) as slabpool,
            tc.tile_pool(name="gixp", bufs=8) as gixpool,
            tc.tile_pool(name="psum", bufs=7, space="PSUM") as pspool,
        ):
            w_sb = cpool.tile([128, K * Cout], BF16, tag="w")
            nc.sync.dma_start(out=w_sb[:, :], in_=wt[:, :])
            zed = cpool.tile([128, 6400], FP16, tag="zed")
            nc.vector.memset(zed[:, :], 0.0)
            zrows = 128 * 6400 // 128  # 6400 rows per DMA
            for bank in accs:
                r0 = 0
                while r0 < acc_total:
                    rcnt = min(zrows, acc_total - r0)
                    nc.sync.dma_start(
                        out=bank[r0:r0 + rcnt, :],
                        in_=zed[:, :rcnt],
                    )
                    r0 += rcnt

            # software pipelining: emit gathers LOOKAHEAD windows ahead of
            # their scatters so the in-order Pool queue never head-of-line
            # blocks on a scatter whose slab is still being produced
            LOOKAHEAD = 5
            pend_scat = []  # (wi, gi, slab, ntile, nw)

            def emit_scatter(ent):
                wi, gi, slab, ntile, nw, n = ent
                nc.gpsimd.dma_scatter_add(
                    out_ap=accs[wi % NBANKS][:, 0:64],
                    in_ap=slab[:, :ntile, :],
                    idxs_ap=gi[:, nw:2 * nw],
                    num_idxs=n,
                    num_idxs_reg=n,
                    elem_size=64,
                    elem_step=128,
                    single_packet=SINGLE_PACKET,
                )

            for wi, w in enumerate(windows):
                n = w["n"]
                ntile = n // 128
                nw = n // 16
                gi = gixpool.tile([128, 2 * (SEG // 16)], I16, tag="gi")
                nc.sync.dma_start(
                    out=gi[:, :2 * nw],
                    in_=gsidx[:, 2 * w["gcol0"]:2 * w["gcol0"] + 2 * nw],
                )
                g = gpool.tile([128, 1, SEG], BF16, tag="g")
                c = w["c"]
                nc.gpsimd.dma_gather(
                    out_ap=g[:, :, :n],
                    in_ap=ftab[c * CHUNK:min((c + 1) * CHUNK, ftab_rows), :],
                    idxs_ap=gi[:, :nw],
                    num_idxs=n,
                    num_idxs_reg=n,
                    elem_size=128,
                    transpose=True,
                    single_packet=SINGLE_PACKET,
                )
                slab = slabpool.tile([128, SEG // 128, 64], FP16, tag="slab")
                # PSUM sub-blocks of 8 slab tiles (1024 slots = one bank)
                for t0 in range(0, ntile, 8):
                    nt8 = min(8, ntile - t0)
                    ps = pspool.tile([128, 8, 64], F32, tag="ps")
                    for (k, off, ln) in w["pieces"]:
                        t = off // 128
                        if not (t0 <= t < t0 + 8):
                            continue
                        p0 = off % 128
                        nc.tensor.matmul(
                            out=ps[p0:p0 + ln, t - t0, :],
                            lhsT=g[:, 0, off:off + ln],
                            rhs=w_sb[:, k * Cout:(k + 1) * Cout],
                            start=True, stop=True,
                        )
                    nc.vector.tensor_copy(
                        out=slab[:, t0:t0 + nt8, :], in_=ps[:, :nt8, :])
                pend_scat.append((wi, gi, slab, ntile, nw, n))
                if len(pend_scat) > LOOKAHEAD:
                    emit_scatter(pend_scat.pop(0))
            for ent in pend_scat:
                emit_scatter(ent)

        # ---- BN phase ----
        Tp = _roundup(Tb, 8)  # stat tiles padded so all matmuls are 512-wide
        F32R = mybir.dt.float32r
        with (
            tc.tile_pool(name="bn", bufs=1) as bnpool,
            tc.tile_pool(name="bns", bufs=4) as bnspool,
            tc.tile_pool(name="bnp", bufs=2, space="PSUM") as bnps,
        ):
            out_sb = bnpool.tile([128, Tp, 64], F32, tag="outsb")
            gam_t = bnspool.tile([1, 64], F32, tag="gam")
            bet_t = bnspool.tile([1, 64], F32, tag="bet")
            nc.sync.dma_start(out=gam_t[:, :], in_=gb[0:1, :])
            nc.sync.dma_start(out=bet_t[:, :], in_=gb[1:2, :])
            ones16 = bnpool.tile([128, 1], FP16, tag="ones16")
            nc.vector.memset(ones16[:, :], 1.0)
            onesb = bnpool.tile([128, 1], BF16, tag="onesb")
            nc.vector.memset(onesb[:, :], 1.0)
            sum_ps = bnps.tile([1, 512], F32, tag="sum")
            sq_ps = bnps.tile([1, 512], F32, tag="sq")
            if Tp > Tb:
                nc.vector.memset(out_sb[:, Tb:Tp, :], 0.0)
            # per-channel sums read the fp16 banks directly, pipelining each
            # bank's sum matmuls under the next bank's load
            ngrp_b = (Tb + 7) // 8  # 25 groups over Tb (last ragged)
            nsum = NBANKS * ngrp_b
            si = 0
            with tc.tile_pool(name="bnb", bufs=2) as bnbpool:
                bts = []
                for b in range(NBANKS):
                    bt = bnbpool.tile([128, Tb, 128], FP16, tag="bank")
                    nc.sync.dma_start(out=bt[:, :, :],
                                      in_=accs[b][0:acc_rows, :])
                    bts.append(bt)
                    for i in range(ngrp_b):
                        nt = min(8, Tb - 8 * i)
                        nc.tensor.matmul(
                            out=sum_ps[:, :nt * 64], lhsT=ones16[:, :],
                            rhs=bt[:, 8 * i:8 * i + nt, 0:64],
                            start=(si == 0), stop=(si == nsum - 1),
                            skip_group_check=True,
                        )
                        si += 1
                nc.vector.tensor_tensor(
                    out=out_sb[:, 0:Tb, :],
                    in0=bts[0][:, :, 0:64], in1=bts[1][:, :, 0:64],
                    op=mybir.AluOpType.add)
            ngrp = Tp // 8
            for i in range(ngrp):
                sqt = bnspool.tile([128, 8, 64], BF16, tag="sqt")
                nc.vector.tensor_tensor(
                    out=sqt[:, :, :], in0=out_sb[:, 8 * i:8 * i + 8, :],
                    in1=out_sb[:, 8 * i:8 * i + 8, :],
                    op=mybir.AluOpType.mult)
                nc.tensor.matmul(
                    out=sq_ps[:, :], lhsT=onesb[:, :], rhs=sqt[:, :, :],
                    start=(i == 0), stop=(i == ngrp - 1),
                    skip_group_check=True,
                )
            st0 = bnspool.tile([1, 512], F32, tag="st0")
            st1 = bnspool.tile([1, 512], F32, tag="st1")
            nc.vector.tensor_copy(out=st0[:, :], in_=sum_ps[:, :])
            nc.vector.tensor_copy(out=st1[:, :], in_=sq_ps[:, :])
            for st in (st0, st1):
                nc.vector.tensor_tensor(
                    out=st[:, 0:256], in0=st[:, 0:256], in1=st[:, 256:512],
                    op=mybir.AluOpType.add)
                nc.vector.tensor_tensor(
                    out=st[:, 0:128], in0=st[:, 0:128], in1=st[:, 128:256],
                    op=mybir.AluOpType.add)
                nc.vector.tensor_tensor(
                    out=st[:, 0:64], in0=st[:, 0:64], in1=st[:, 64:128],
                    op=mybir.AluOpType.add)
            nc.sync.dma_start(out=cc_in[0:1, :], in_=st0[:, 0:64])
            nc.sync.dma_start(out=cc_in[1:2, :], in_=st1[:, 0:64])
            nc.gpsimd.collective_compute(
                "AllReduce",
                mybir.AluOpType.add,
                ins=[cc_in[:, :]],
                outs=[cc_out[:, :]],
                replica_groups=[list(range(n_cores))],
            )
            gs0 = bnspool.tile([1, 64], F32, tag="gs0")
            gs1 = bnspool.tile([1, 64], F32, tag="gs1")
            nc.sync.dma_start(out=gs0[:, :], in_=cc_out[0:1, :])
            nc.sync.dma_start(out=gs1[:, :], in_=cc_out[1:2, :])

            inv_n = 1.0 / float(n_out)
            mean_t = bnspool.tile([1, 64], F32, tag="mean")
            ex2_t = bnspool.tile([1, 64], F32, tag="ex2")
            var_t = bnspool.tile([1, 64], F32, tag="var")
            sd_t = bnspool.tile([1, 64], F32, tag="sd")
            rs_t = bnspool.tile([1, 64], F32, tag="rs")
            a_t = bnspool.tile([1, 64], F32, tag="a")
            b_t = bnspool.tile([1, 64], F32, tag="b")
            nc.vector.tensor_scalar_mul(mean_t[:, :], gs0[:, :], inv_n)
            nc.vector.tensor_scalar_mul(ex2_t[:, :], gs1[:, :], inv_n)
            nc.vector.tensor_tensor(
                out=var_t[:, :], in0=mean_t[:, :], in1=mean_t[:, :],
                op=mybir.AluOpType.mult)
            nc.vector.tensor_tensor(
                out=var_t[:, :], in0=ex2_t[:, :], in1=var_t[:, :],
                op=mybir.AluOpType.subtract)
            nc.vector.tensor_scalar_add(var_t[:, :], var_t[:, :], BN_EPS)
            nc.scalar.activation(
                out=sd_t[:, :], in_=var_t[:, :],
                func=mybir.ActivationFunctionType.Sqrt)
            nc.vector.reciprocal(out=rs_t[:, :], in_=sd_t[:, :])
            nc.vector.tensor_tensor(
                out=a_t[:, :], in0=gam_t[:, :], in1=rs_t[:, :],
                op=mybir.AluOpType.mult)
            nc.vector.tensor_tensor(
                out=b_t[:, :], in0=mean_t[:, :], in1=a_t[:, :],
                op=mybir.AluOpType.mult)
            nc.vector.tensor_tensor(
                out=b_t[:, :], in0=bet_t[:, :], in1=b_t[:, :],
                op=mybir.AluOpType.subtract)
            # broadcast [1,64] -> [128,64] via PE (ones[128,1] @ row)
            ones_row = bnspool.tile([1, 128], F32, tag="ones_row")
            nc.vector.memset(ones_row[:, :], 1.0)
            a_full = bnspool.tile([128, 64], F32, tag="afull")
            b_full = bnspool.tile([128, 64], F32, tag="bfull")
            ab_ps = bnps.tile([128, 64], F32, tag="abps")
            nc.tensor.matmul(
                out=ab_ps[:, :], lhsT=ones_row[:, :], rhs=a_t[:, :],
                start=True, stop=True)
            nc.vector.tensor_copy(out=a_full[:, :], in_=ab_ps[:, :])
            nc.tensor.matmul(
                out=ab_ps[:, :], lhsT=ones_row[:, :], rhs=b_t[:, :],
                start=True, stop=True)
            nc.vector.tensor_copy(out=b_full[:, :], in_=ab_ps[:, :])
            # fused normalize + relu over row halves, each half's y write
            # overlapping the next half's compute
            half = Tb // 2
            for (t0, t1) in ((0, half), (half, Tb)):
                nt = t1 - t0
                nc.vector.tensor_tensor(
                    out=out_sb[:, t0:t1, :], in0=out_sb[:, t0:t1, :],
                    in1=a_full[:, None, :].to_broadcast((128, nt, 64)),
                    op=mybir.AluOpType.mult)
                nc.vector.tensor_tensor(
                    out=out_sb[:, t0:t1, :], in0=out_sb[:, t0:t1, :],
                    in1=b_full[:, None, :].to_broadcast((128, nt, 64)),
                    op=mybir.AluOpType.add)
                nc.scalar.activation(
                    out=out_sb[:, t0:t1, :], in_=out_sb[:, t0:t1, :],
                    func=mybir.ActivationFunctionType.Relu)
                nc.sync.dma_start(
                    out=y[:, t0:t1, :], in_=out_sb[:, t0:t1, :])

    nc.compile()
    return nc


def _prepare(feats, W, gamma, beta, in_map, out_map, n_out, n_cores):
    """Host prep shared by kernel() and tests.  Returns (nc, in_maps, plan)."""
    n_out = int(n_out)
    K, Cin, Cout = W.shape
    assert Cin == 64 and Cout == 64
    in_map = np.asarray(in_map, dtype=np.int64)
    out_map = np.asarray(out_map, dtype=np.int64)
    feats = np.asarray(feats, dtype=np.float32)
    W = np.asarray(W, dtype=np.float32)

    plan, gsidx_all = _route(in_map, out_map, n_out, n_cores)

    ftab_rows = _roundup(feats.shape[0], CHUNK)
    ftab = np.zeros((ftab_rows, 128), dtype=ml_dtypes.bfloat16)
    ftab[:feats.shape[0], :64] = feats.astype(ml_dtypes.bfloat16)

    wt = np.zeros((128, K * 64), dtype=ml_dtypes.bfloat16)
    wt[:64, :] = (
        W.transpose(1, 0, 2).reshape(64, K * 64).astype(ml_dtypes.bfloat16))

    gb = np.stack([np.asarray(gamma, np.float32),
                   np.asarray(beta, np.float32)])

    nc = _build(plan, n_out, ftab_rows, n_cores)
    in_maps = [
        dict(ftab=ftab, wt=wt, gsidx=gsidx_all[c], gb=gb)
        for c in range(n_cores)
    ]
    return nc, in_maps, plan


def kernel(feats, W, gamma, beta, in_map, out_map, n_out):
    from concourse.bass_utils import run_bass_kernel_spmd

    n_cores = 8
    nc, in_maps, plan = _prepare(
        feats, W, gamma, beta, in_map, out_map, n_out, n_cores)
    res = run_bass_kernel_spmd(nc, in_maps, list(range(n_cores)))
    rows = plan["rows_per_core"]
    out = np.concatenate(
        [np.asarray(res.results[c]["y"]).reshape(-1, 64)[:rows]
         for c in range(n_cores)], axis=0)
    return out.astype(np.float32)


# revision 68
# speedup vs baseline: 3.0904x; 1.1031x over previous
"""Trainium2 Bass kernel for nn_BasicDeconvolutionBlock.

Reference computation:
    gathered = feats[in_map]                         # [K, M, Cin]
    contrib  = einsum('kmc,kcd->kmd', gathered, W)   # [K, M, Cout]
    out      = zeros([n_out, Cout]).at[out_map].add(contrib)
    y        = relu(batchnorm(out))                  # batch stats over n_out rows

Strategy (8 NeuronCores, SPMD):
  - Host routes each (k, m) pair to the core owning its output row
    (row blocks of n_out/8).  Per core ~169k pairs.
  - Slot stream per core: for each feats chunk (int16 gather range, 32768
    rows), pairs are laid out in (stage, k) groups 32-aligned (shared
    caps = max pair count over cores; ragged matmul pieces handle group
    boundaries).  The stream is cut into SEG-slot scatter windows; pairs
    of gather-adjacent windows share one big dma_gather call of SEG_G
    slots.  single_packet=False lets SWDGE calls exceed the 1024-index
    limit that wedges the Q7 ucode in single-packet mode; the per-call
    994ns fixed cost then amortizes over many more slots.
  - Duplicate out-rows inside one scatter call race in hardware, so the
    host EVICTS same-(window,row) duplicates to overflow stages appended
    to the same chunk run (iterated until dup-free, with cross-stage
    used-(window,row) tracking).  Cross-window dups are safe: same-bank
    windows serialize via Tile WAW deps, and the two HBM accumulator
    banks alternate by window parity so adjacent windows overlap.
  - Gather: feats pre-cast to bf16, padded to 128 channels (256B rows);
    dma_gather(transpose=True) yields channel-major G[128, slots].
    Gathers are prefetched two blocks ahead; scatters trail their window
    by LOOKAHEAD so the in-order Pool queue never head-of-line blocks.
  - GEMM: ragged pieces (cut at k-group/128-tile boundaries, PSUM base
    partitions 0/32/64 only; a 96-start run is covered by a wide base-64
    matmul emitted first then overwritten) into per-1024-slot PSUM banks
    (slot-major: partition=slot%128), one DVE fp16 copy per bank.
  - Scatter: dma_scatter_add (CCE add, fp16, elem 64, row stride 256B)
    into acc banks [acc_rows+128, 128] fp16 (cols 64:128 unused pad to
    satisfy the 256B row-stride requirement); pad/hole/evicted slots go
    to a dump row beyond acc_rows.
  - BN: banks stream in as partition halves with per-channel sum matmuls
    (fp16, direct from banks) pipelined under the loads; halves fold to
    a bf16 x; sum of squares via bf16 group tiles; [2,64] AllReduce
    across 8 cores; bf16 normalize in row quarters with f32 relu
    staging, each quarter's y write overlapping the next's compute.
"""

import numpy as np

import sys

sys.path.insert(0, "/opt/trn_rl_repo")

import ml_dtypes  # noqa: E402

from concourse import bacc, bass, mybir  # noqa: E402
import concourse.tile as tile  # noqa: E402

import os

BN_EPS = 1e-5
CHUNK = 32768  # int16 gather index range per feats chunk
SEG = int(os.environ.get("DECONV_SEG", "1408"))  # scatter window slots
SEG_G = int(os.environ.get("DECONV_SEG_G", "2816"))  # gather call slots
SINGLE_PACKET = os.environ.get("DECONV_SP", "0") == "1"
NBANKS = int(os.environ.get("DECONV_NBANKS", "2"))
F32 = mybir.dt.float32
FP16 = mybir.dt.float16
BF16 = mybir.dt.bfloat16
I16 = mybir.dt.int16


def _roundup(x, m):
    return (x + m - 1) // m * m


def _occ_rank(keys):
    """Rank of each element among equal values of `keys` (stable)."""
    order = np.argsort(keys, kind="stable")
    ks = keys[order]
    n = len(ks)
    first = np.ones(n, dtype=bool)
    if n:
        first[1:] = ks[1:] != ks[:-1]
    grp = np.maximum.accumulate(np.where(first, np.arange(n), 0))
    rank_sorted = np.arange(n) - grp
    out = np.empty(n, dtype=np.int64)
    out[order] = rank_sorted
    return out


def _route(in_map, out_map, n_out, n_cores):
    """Host-side routing.  Returns (plan, gidx_all, sidx_all).

    plan.windows: list of dicts (c, slot0, n, gcol0, pieces=[(k, off, len)])
    where slot0/gcol0 are global and off is window-relative.
    """
    K, M = in_map.shape
    rows_per_core = n_out // n_cores
    assert rows_per_core * n_cores == n_out
    acc_rows = _roundup(rows_per_core, 128)
    dump_row = acc_rows
    nchunk = _roundup(int(in_map.max()) + 1, CHUNK) // CHUNK

    k_idx = np.repeat(np.arange(K, dtype=np.int64), M)
    in_flat = in_map.ravel().astype(np.int64)
    out_flat = out_map.ravel().astype(np.int64)
    core_of = out_flat // rows_per_core
    row_local = out_flat % rows_per_core
    chunk_of = in_flat // CHUNK
    idx_local = in_flat % CHUNK

    # pending pairs per (core, chunk)
    pend = {}
    for ci in range(n_cores):
        sel = np.nonzero(core_of == ci)[0]
        for c in range(nchunk):
            s2 = sel[chunk_of[sel] == c]
            pend[(ci, c)] = (k_idx[s2], idx_local[s2], row_local[s2])

    # iterate stages per chunk until dup-free; build shared group layout
    run_kk = [[] for _ in range(nchunk)]  # per-chunk per-slot k (shared)
    run_len = np.zeros(nchunk, dtype=np.int64)
    # per (core, chunk): placed slot -> (gidx value, sidx value)
    placed = {key: ([], [], []) for key in pend}  # slots, gvals, svals

    for c in range(nchunk):
        stage = 0
        no_progress = False
        prev_pend = None
        used = {ci: np.empty(0, dtype=np.int64) for ci in range(n_cores)}
        while True:
            counts = np.zeros((n_cores, K), dtype=np.int64)
            for ci in range(n_cores):
                kk, _, _ = pend[(ci, c)]
                if len(kk):
                    np.add.at(counts[ci], kk, 1)
            caps = counts.max(axis=0)
            # exact caps (no alignment): the piece builder widens each
            # run to a legal PSUM base (0/32/64) and emits tiles
            # right-to-left so true-k pieces overwrite the overhang
            if caps.sum() == 0:
                break
            assert stage < 60, "eviction did not converge"
            if no_progress:
                # previous stage stalled: start on a fresh window so
                # high-multiplicity rows always find a free (window,row)
                tgt = _roundup(int(run_len[c]), SEG)
                if tgt > run_len[c]:
                    lastk = run_kk[c][-1] if run_kk[c] else 0
                    run_kk[c].extend([lastk] * int(tgt - run_len[c]))
                    run_len[c] = tgt
            cur_pend = sum(len(pend[(ci, c)][0]) for ci in range(n_cores))
            no_progress = prev_pend is not None and cur_pend >= prev_pend
            prev_pend = cur_pend
            g0 = np.zeros(K + 1, dtype=np.int64)
            g0[1:] = np.cumsum(caps)
            base = run_len[c]
            for k in range(K):
                run_kk[c].extend([k] * int(caps[k]))
            run_len[c] += caps.sum()
            for ci in range(n_cores):
                kk, iv, rv = pend[(ci, c)]
                if not len(kk):
                    pend[(ci, c)] = (kk, iv, rv)
                    continue
                # in-group order: (occ among same (k,row), row) to spread
                # a row's duplicates across the group's windows
                occ = _occ_rank(kk * rows_per_core + rv)
                order = np.lexsort((rv, occ, kk))
                kk, iv, rv = kk[order], iv[order], rv[order]
                starts = np.concatenate(
                    [[0], np.cumsum(np.bincount(kk, minlength=K))])
                rank = np.arange(len(kk)) - starts[kk]
                slot = base + g0[kk] + rank
                win = slot // SEG
                key = win * rows_per_core + rv
                # keep the first pair (in provisional order) per (win,row),
                # excluding (win,row) pairs already used by earlier stages
                fresh = ~np.isin(key, used[ci])
                order2 = np.lexsort((np.arange(len(key)), key))
                ks = key[order2]
                first = np.ones(len(ks), dtype=bool)
                if len(ks):
                    first[1:] = ks[1:] != ks[:-1]
                keep = np.zeros(len(key), dtype=bool)
                keep[order2[first]] = True
                keep &= fresh
                used[ci] = np.concatenate([used[ci], key[keep]])
                ps, pg, pv = placed[(ci, c)]
                ps.append(slot[keep])
                pg.append(iv[keep])
                pv.append(rv[keep])
                # hole repair: retry each evicted pair in its own group's
                # free slots (32-align tail + previously vacated slots) in
                # windows that don't already hold its row -- same k, so the
                # shared matmul piece map is unchanged
                evict_idx = np.nonzero(~keep)[0]
                if len(evict_idx):
                    usedset = set(int(x) for x in used[ci])
                    cnts = np.bincount(kk, minlength=K)
                    holes = {
                        kq: list(range(int(base + g0[kq] + cnts[kq]),
                                       int(base + g0[kq] + caps[kq])))
                        for kq in range(K) if caps[kq] > cnts[kq]
                    }
                    rep_s, rep_g, rep_r, still = [], [], [], []
                    for i in evict_idx:
                        kq, r = int(kk[i]), int(rv[i])
                        hl = holes.get(kq)
                        ok = False
                        if hl:
                            for hi in range(len(hl)):
                                h = hl[hi]
                                hkey = (h // SEG) * rows_per_core + r
                                if hkey not in usedset:
                                    usedset.add(hkey)
                                    rep_s.append(h)
                                    rep_g.append(int(iv[i]))
                                    rep_r.append(r)
                                    hl[hi] = int(slot[i])
                                    ok = True
                                    break
                        if not ok:
                            still.append(i)
                            # its vacated slot is a valid hole for OTHER rows
                            holes.setdefault(kq, []).append(int(slot[i]))
                    if rep_s:
                        ps.append(np.array(rep_s, dtype=np.int64))
                        pg.append(np.array(rep_g, dtype=np.int64))
                        pv.append(np.array(rep_r, dtype=np.int64))
                    rem = np.array(still, dtype=np.int64)
                    pend[(ci, c)] = (kk[rem], iv[rem], rv[rem])
                    used[ci] = np.fromiter(
                        usedset, dtype=np.int64, count=len(usedset))
                else:
                    pend[(ci, c)] = (
                        kk[~keep], iv[~keep], rv[~keep])
            stage += 1

    # pad runs to 128; build windows and global offsets
    run_pad = np.array([_roundup(int(x), 128) for x in run_len], dtype=np.int64)
    for c in range(nchunk):
        lastk = run_kk[c][-1] if run_kk[c] else 0
        run_kk[c].extend([lastk] * int(run_pad[c] - run_len[c]))
    run0 = np.zeros(nchunk + 1, dtype=np.int64)
    run0[1:] = np.cumsum(run_pad)
    total_slots = int(run0[-1])

    windows = []
    for c in range(nchunk):
        kkarr = np.array(run_kk[c], dtype=np.int64)
        off = 0
        while off < run_pad[c]:
            n = int(min(SEG, run_pad[c] - off))
            # pieces: k-runs cut at 128-tile boundaries.  PE tile-position
            # rules allow PSUM bases 0/32/64 only (len <=128/32/64), so
            # each run is WIDENED left to a legal base and tiles emit
            # right-to-left: a later-emitted (earlier) run's piece
            # overwrites the overhang it owns (start=True resets the
            # written region), leaving every slot's final writer = true k.
            tile_runs = {}
            j = 0
            while j < n:
                k = int(kkarr[off + j])
                e = j + 1
                while e < n and kkarr[off + e] == k:
                    e += 1
                s = j
                while s < e:
                    t = s // 128
                    te = min(e, (t + 1) * 128)
                    tile_runs.setdefault(t, []).append((k, s, te))
                    s = te
                j = e
            pieces = []
            for t in sorted(tile_runs):
                tile_ps = []
                for (k, s, e) in tile_runs[t]:
                    sl, el = s - t * 128, e - t * 128
                    for b in (64, 32, 0):
                        lim = 128 if b == 0 else (64 if b == 64 else 32)
                        if b <= sl and el - b <= lim:
                            break
                    tile_ps.append((k, t * 128 + b, el - b))
                pieces.extend(reversed(tile_ps))
            windows.append(
                dict(
                    c=c,
                    slot0=int(run0[c] + off),
                    n=n,
                    gcol0=int((run0[c] + off) // 16),
                    pieces=pieces,
                )
            )
            off += n

    # gather calls: big runs (up to SEG_G) covering whole scatter windows
    # within one chunk, amortizing the SWDGE fixed cost over many windows
    gcalls = []
    i = 0
    while i < len(windows):
        c = windows[i]["c"]
        j = i
        n = 0
        while j < len(windows) and windows[j]["c"] == c and \
                n + windows[j]["n"] <= SEG_G:
            n += windows[j]["n"]
            j += 1
        gcalls.append(dict(c=c, slot0=windows[i]["slot0"], n=n,
                           wlo=i, whi=j))
        i = j

    # per-core packed idx arrays (16-wrapped, replicated to 128 partitions)
    gcols = total_slots // 16
    gidx_all = np.zeros((n_cores, 128, gcols), dtype=np.int16)
    sidx_all = np.empty((n_cores, 128, gcols), dtype=np.int16)
    for ci in range(n_cores):
        gvals = np.zeros(total_slots, dtype=np.int16)
        svals = np.full(total_slots, dump_row, dtype=np.int16)
        for c in range(nchunk):
            ps, pg, pv = placed[(ci, c)]
            if not ps:
                continue
            slots = np.concatenate(ps) + run0[c]
            gvals[slots] = np.concatenate(pg).astype(np.int16)
            svals[slots] = np.concatenate(pv).astype(np.int16)
        gidx_all[ci] = np.tile(gvals.reshape(-1, 16).T, (8, 1))
        sidx_all[ci] = np.tile(svals.reshape(-1, 16).T, (8, 1))

    plan = dict(
        nchunk=nchunk,
        rows_per_core=rows_per_core,
        acc_rows=acc_rows,
        acc_total=acc_rows + 128,
        dump_row=dump_row,
        windows=windows,
        gcalls=gcalls,
        total_slots=total_slots,
        gcols=gcols,
    )
    return plan, gidx_all, sidx_all


def _build(plan, n_out, ftab_rows, n_cores):
    """Trace the Bass program.  Returns nc."""
    nc = bacc.Bacc("TRN2", target_bir_lowering=False, debug=False)

    K = 27
    Cout = 64
    acc_rows, acc_total = plan["acc_rows"], plan["acc_total"]
    windows = plan["windows"]
    gcalls = plan["gcalls"]
    Tb = acc_rows // 128  # BN column tiles

    ftab = nc.dram_tensor("ftab", [ftab_rows, 128], BF16, kind="ExternalInput")
    wt = nc.dram_tensor("wt", [128, K * Cout], BF16, kind="ExternalInput")
    gidx = nc.dram_tensor("gidx", [128, plan["gcols"]], I16,
                          kind="ExternalInput")
    sidx = nc.dram_tensor("sidx", [128, plan["gcols"]], I16,
                          kind="ExternalInput")
    gb = nc.dram_tensor("gb", [2, Cout], F32, kind="ExternalInput")
    accs = [
        nc.dram_tensor(f"acc{b}", [acc_total, 128], FP16) for b in range(NBANKS)
    ]
    cc_in = nc.dram_tensor("cc_in", [2, Cout], F32)
    cc_out = nc.dram_tensor("cc_out", [2, Cout], F32, addr_space="Shared")
    # 3D view of the [acc_rows, 64] output (row = p*Tb + t) so row-half
    # writes can overlap the normalize pipeline
    y = nc.dram_tensor("y", [128, acc_rows // 128, Cout], FP16,
                       kind="ExternalOutput")

    with tile.TileContext(nc) as tc:
        with (
            tc.tile_pool(name="const", bufs=1) as cpool,
            tc.tile_pool(name="gpool", bufs=int(os.environ.get("DECONV_GBUFS", "3"))) as gpool,
            tc.tile_pool(name="slab", bufs=8) as slabpool,
            tc.tile_pool(name="gixp", bufs=int(os.environ.get("DECONV_GBUFS", "3"))) as gixpool,
            tc.tile_pool(name="sixp", bufs=8) as sixpool,
            tc.tile_pool(name="psum", bufs=7, space="PSUM") as pspool,
        ):
            w_sb = cpool.tile([128, K * Cout], BF16, tag="w")
            nc.sync.dma_start(out=w_sb[:, :], in_=wt[:, :])
            zed = cpool.tile([128, 6400], FP16, tag="zed")
            nc.vector.memset(zed[:, :], 0.0)

            def emit_zero_init():
                # acc zero-init, emitted AFTER the first windows' gathers so
                # the early idx loads aren't queued behind 13 MB of init DMA
                # (must still precede the first scatter in program order for
                # Tile's WAW ordering)
                zrows = 6400
                for bank in accs:
                    r0 = 0
                    while r0 < acc_total:
                        rcnt = min(zrows, acc_total - r0)
                        nc.sync.dma_start(
                            out=bank[r0:r0 + rcnt, :],
                            in_=zed[:, :rcnt],
                        )
                        r0 += rcnt

            # Each gather call covers several scatter windows.  Software
            # pipelining: scatters trail their window by LOOKAHEAD windows
            # and the next block's gather is emitted before them, so the
            # in-order Pool queue never head-of-line blocks on a scatter
            # whose slab is still being produced.
            LOOKAHEAD = 5
            pend_scat = []

            def emit_scatter(ent):
                wi, si_t, slab, ntile, nw, n = ent
                nc.gpsimd.dma_scatter_add(
                    out_ap=accs[wi % NBANKS][:, 0:64],
                    in_ap=slab[:, :ntile, :],
                    idxs_ap=si_t,
                    num_idxs=n,
                    num_idxs_reg=n,
                    elem_size=64,
                    elem_step=128,
                    single_packet=SINGLE_PACKET,
                )

            def emit_gather(gc):
                ng = gc["n"]
                c = gc["c"]
                gcol0 = gc["slot0"] // 16
                gi = gixpool.tile([128, SEG_G // 16], I16, tag="gi")
                nc.sync.dma_start(
                    out=gi[:, :ng // 16],
                    in_=gidx[:, gcol0:gcol0 + ng // 16],
                )
                g = gpool.tile([128, 1, SEG_G], BF16, tag="g")
                nc.gpsimd.dma_gather(
                    out_ap=g[:, :, :ng],
                    in_ap=ftab[c * CHUNK:min((c + 1) * CHUNK, ftab_rows), :],
                    idxs_ap=gi[:, :ng // 16],
                    num_idxs=ng,
                    num_idxs_reg=ng,
                    elem_size=128,
                    transpose=True,
                    single_packet=SINGLE_PACKET,
                )
                return g

            # prefetch gathers two blocks ahead of their windows
            gts = {}
            for j in range(min(2, len(gcalls))):
                gts[j] = emit_gather(gcalls[j])
            for gj, gc in enumerate(gcalls):
                g = gts.pop(gj)
                if gj + 2 < len(gcalls):
                    gts[gj + 2] = emit_gather(gcalls[gj + 2])
                # one scatter-idx load covers the whole block (consecutive
                # windows have contiguous sidx columns)
                bcol0 = gc["slot0"] // 16
                bcols = gc["n"] // 16
                si_b = sixpool.tile([128, SEG_G // 16], I16, tag="si")
                nc.sync.dma_start(
                    out=si_b[:, :bcols],
                    in_=sidx[:, bcol0:bcol0 + bcols],
                )
                for wi in range(gc["wlo"], gc["whi"]):
                    w = windows[wi]
                    n = w["n"]
                    ntile = n // 128
                    nw = n // 16
                    woff = w["slot0"] - gc["slot0"]  # window base within g
                    si_t = si_b[:, woff // 16:woff // 16 + nw]
                    slab = slabpool.tile([128, SEG // 128, 64], FP16,
                                         tag="slab")
                    # PSUM sub-blocks of 8 slab tiles (1024 slots = one bank)
                    for t0 in range(0, ntile, 8):
                        nt8 = min(8, ntile - t0)
                        ps = pspool.tile([128, 8, 64], F32, tag="ps")
                        for (k, off, ln) in w["pieces"]:
                            t = off // 128
                            if not (t0 <= t < t0 + 8):
                                continue
                            p0 = off % 128
                            nc.tensor.matmul(
                                out=ps[p0:p0 + ln, t - t0, :],
                                lhsT=g[:, 0, woff + off:woff + off + ln],
                                rhs=w_sb[:, k * Cout:(k + 1) * Cout],
                                start=True, stop=True,
                            )
                        nc.vector.tensor_copy(
                            out=slab[:, t0:t0 + nt8, :], in_=ps[:, :nt8, :])
                    pend_scat.append((wi, si_t, slab, ntile, nw, n))
                    if wi == min(1, LOOKAHEAD - 1):
                        emit_zero_init()
                    if len(pend_scat) > LOOKAHEAD:
                        emit_scatter(pend_scat.pop(0))
            for ent in pend_scat:
                emit_scatter(ent)

        # ---- BN phase ----
        Tp = _roundup(Tb, 8)  # stat tiles padded so all matmuls are 512-wide
        F32R = mybir.dt.float32r
        with (
            tc.tile_pool(name="bn", bufs=1) as bnpool,
            tc.tile_pool(name="bns", bufs=4) as bnspool,
            tc.tile_pool(name="bnp", bufs=2, space="PSUM") as bnps,
        ):
            xb = bnpool.tile([128, Tp, 64], BF16, tag="xb")
            gam_t = bnspool.tile([1, 64], F32, tag="gam")
            bet_t = bnspool.tile([1, 64], F32, tag="bet")
            nc.sync.dma_start(out=gam_t[:, :], in_=gb[0:1, :])
            nc.sync.dma_start(out=bet_t[:, :], in_=gb[1:2, :])
            ones16 = bnpool.tile([128, 1], FP16, tag="ones16")
            nc.vector.memset(ones16[:, :], 1.0)
            onesb = bnpool.tile([128, 1], BF16, tag="onesb")
            nc.vector.memset(onesb[:, :], 1.0)
            sum_ps = bnps.tile([1, 512], F32, tag="sum")
            sq_ps = bnps.tile([1, 512], F32, tag="sq")
            if Tp > Tb:
                nc.vector.memset(xb[:, Tb:Tp, :], 0.0)
            # banks stream in as partition halves (rows p*Tb+t, so half the
            # rows = partitions 0:64); per-channel sums read the fp16 banks
            # directly and each half folds (to bf16) as soon as its pair
            # lands, all pipelining under the remaining load DMAs
            ngrp_b = (Tb + 7) // 8  # 25 groups over Tb (last ragged)
            nsum = 2 * NBANKS * ngrp_b
            rows_half = 64 * Tb
            si = 0
            with tc.tile_pool(name="bnb", bufs=2) as bnbpool:
                bank0_sb = bnbpool.tile([128, Tb, 128], FP16, tag="bank")
                bank1_sb = bnbpool.tile([128, Tb, 128], FP16, tag="bank")
                bts = [bank0_sb, bank1_sb]
                for b in range(NBANKS):
                    for h in range(2):
                        p0, p1 = h * 64, (h + 1) * 64
                        nc.sync.dma_start(
                            out=bts[b][p0:p1, :, :],
                            in_=accs[b][h * rows_half:(h + 1) * rows_half, :])
                        for i in range(ngrp_b):
                            nt = min(8, Tb - 8 * i)
                            nc.tensor.matmul(
                                out=sum_ps[:, :nt * 64],
                                lhsT=ones16[p0:p1, :],
                                rhs=bts[b][p0:p1, 8 * i:8 * i + nt, 0:64],
                                start=(si == 0), stop=(si == nsum - 1),
                                skip_group_check=True,
                            )
                            si += 1
                        if b == NBANKS - 1:
                            nc.vector.tensor_tensor(
                                out=xb[p0:p1, 0:Tb, :],
                                in0=bts[0][p0:p1, :, 0:64],
                                in1=bts[1][p0:p1, :, 0:64],
                                op=mybir.AluOpType.add)
            ngrp = Tp // 8
            for i in range(ngrp):
                sqt = bnspool.tile([128, 8, 64], BF16, tag="sqt")
                nc.vector.tensor_tensor(
                    out=sqt[:, :, :], in0=xb[:, 8 * i:8 * i + 8, :],
                    in1=xb[:, 8 * i:8 * i + 8, :],
                    op=mybir.AluOpType.mult)
                nc.tensor.matmul(
                    out=sq_ps[:, :], lhsT=onesb[:, :], rhs=sqt[:, :, :],
                    start=(i == 0), stop=(i == ngrp - 1),
                    skip_group_check=True,
                )
            st0 = bnspool.tile([1, 512], F32, tag="st0")
            st1 = bnspool.tile([1, 512], F32, tag="st1")
            nc.vector.tensor_copy(out=st0[:, :], in_=sum_ps[:, :])
            nc.vector.tensor_copy(out=st1[:, :], in_=sq_ps[:, :])
            for st in (st0, st1):
                nc.vector.tensor_tensor(
                    out=st[:, 0:256], in0=st[:, 0:256], in1=st[:, 256:512],
                    op=mybir.AluOpType.add)
                nc.vector.tensor_tensor(
                    out=st[:, 0:128], in0=st[:, 0:128], in1=st[:, 128:256],
                    op=mybir.AluOpType.add)
                nc.vector.tensor_tensor(
                    out=st[:, 0:64], in0=st[:, 0:64], in1=st[:, 64:128],
                    op=mybir.AluOpType.add)
            nc.sync.dma_start(out=cc_in[0:1, :], in_=st0[:, 0:64])
            nc.sync.dma_start(out=cc_in[1:2, :], in_=st1[:, 0:64])
            nc.gpsimd.collective_compute(
                "AllReduce",
                mybir.AluOpType.add,
                ins=[cc_in[:, :]],
                outs=[cc_out[:, :]],
                replica_groups=[list(range(n_cores))],
            )
            gs0 = bnspool.tile([1, 64], F32, tag="gs0")
            gs1 = bnspool.tile([1, 64], F32, tag="gs1")
            nc.sync.dma_start(out=gs0[:, :], in_=cc_out[0:1, :])
            nc.sync.dma_start(out=gs1[:, :], in_=cc_out[1:2, :])

            inv_n = 1.0 / float(n_out)
            mean_t = bnspool.tile([1, 64], F32, tag="mean")
            ex2_t = bnspool.tile([1, 64], F32, tag="ex2")
            var_t = bnspool.tile([1, 64], F32, tag="var")
            sd_t = bnspool.tile([1, 64], F32, tag="sd")
            rs_t = bnspool.tile([1, 64], F32, tag="rs")
            a_t = bnspool.tile([1, 64], F32, tag="a")
            b_t = bnspool.tile([1, 64], F32, tag="b")
            nc.vector.tensor_scalar_mul(mean_t[:, :], gs0[:, :], inv_n)
            nc.vector.tensor_scalar_mul(ex2_t[:, :], gs1[:, :], inv_n)
            nc.vector.tensor_tensor(
                out=var_t[:, :], in0=mean_t[:, :], in1=mean_t[:, :],
                op=mybir.AluOpType.mult)
            nc.vector.tensor_tensor(
                out=var_t[:, :], in0=ex2_t[:, :], in1=var_t[:, :],
                op=mybir.AluOpType.subtract)
            nc.vector.tensor_scalar_add(var_t[:, :], var_t[:, :], BN_EPS)
            nc.scalar.activation(
                out=sd_t[:, :], in_=var_t[:, :],
                func=mybir.ActivationFunctionType.Sqrt)
            nc.vector.reciprocal(out=rs_t[:, :], in_=sd_t[:, :])
            nc.vector.tensor_tensor(
                out=a_t[:, :], in0=gam_t[:, :], in1=rs_t[:, :],
                op=mybir.AluOpType.mult)
            nc.vector.tensor_tensor(
                out=b_t[:, :], in0=mean_t[:, :], in1=a_t[:, :],
                op=mybir.AluOpType.mult)
            nc.vector.tensor_tensor(
                out=b_t[:, :], in0=bet_t[:, :], in1=b_t[:, :],
                op=mybir.AluOpType.subtract)
            # broadcast [1,64] -> [128,64] via PE (ones[128,1] @ row)
            ones_row = bnspool.tile([1, 128], F32, tag="ones_row")
            nc.vector.memset(ones_row[:, :], 1.0)
            a_full = bnspool.tile([128, 64], BF16, tag="afull")
            b_full = bnspool.tile([128, 64], BF16, tag="bfull")
            ab_ps = bnps.tile([128, 64], F32, tag="abps")
            nc.tensor.matmul(
                out=ab_ps[:, :], lhsT=ones_row[:, :], rhs=a_t[:, :],
                start=True, stop=True)
            nc.vector.tensor_copy(out=a_full[:, :], in_=ab_ps[:, :])
            nc.tensor.matmul(
                out=ab_ps[:, :], lhsT=ones_row[:, :], rhs=b_t[:, :],
                start=True, stop=True)
            nc.vector.tensor_copy(out=b_full[:, :], in_=ab_ps[:, :])
            # bf16 normalize in row quarters; each quarter's relu (f32 out,
            # Act engine) and y write overlap the next quarter's DVE ops
            with tc.tile_pool(name="yst", bufs=2) as ystpool:
                qt = (Tb + 3) // 4
                for t0 in range(0, Tb, qt):
                    t1 = min(t0 + qt, Tb)
                    nt = t1 - t0
                    nc.vector.tensor_tensor(
                        out=xb[:, t0:t1, :], in0=xb[:, t0:t1, :],
                        in1=a_full[:, None, :].to_broadcast((128, nt, 64)),
                        op=mybir.AluOpType.mult)
                    nc.vector.tensor_tensor(
                        out=xb[:, t0:t1, :], in0=xb[:, t0:t1, :],
                        in1=b_full[:, None, :].to_broadcast((128, nt, 64)),
                        op=mybir.AluOpType.add)
                    yst = ystpool.tile([128, qt, 64], FP16, tag="yst")
                    nc.scalar.activation(
                        out=yst[:, :nt, :], in_=xb[:, t0:t1, :],
                        func=mybir.ActivationFunctionType.Relu)
                    nc.sync.dma_start(
                        out=y[:, t0:t1, :], in_=yst[:, :nt, :])

    nc.compile()
    return nc


def _prepare(feats, W, gamma, beta, in_map, out_map, n_out, n_cores):
    """Host prep shared by kernel() and tests.  Returns (nc, in_maps, plan)."""
    n_out = int(n_out)
    K, Cin, Cout = W.shape
    assert Cin == 64 and Cout == 64
    in_map = np.asarray(in_map, dtype=np.int64)
    out_map = np.asarray(out_map, dtype=np.int64)
    feats = np.asarray(feats, dtype=np.float32)
    W = np.asarray(W, dtype=np.float32)

    plan, gidx_all, sidx_all = _route(in_map, out_map, n_out, n_cores)

    ftab_rows = _roundup(feats.shape[0], CHUNK)
    ftab = np.zeros((ftab_rows, 128), dtype=ml_dtypes.bfloat16)
    ftab[:feats.shape[0], :64] = feats.astype(ml_dtypes.bfloat16)

    wt = np.zeros((128, K * 64), dtype=ml_dtypes.bfloat16)
    wt[:64, :] = (
        W.transpose(1, 0, 2).reshape(64, K * 64).astype(ml_dtypes.bfloat16))

    gb = np.stack([np.asarray(gamma, np.float32),
                   np.asarray(beta, np.float32)])

    nc = _build(plan, n_out, ftab_rows, n_cores)
    in_maps = [
        dict(ftab=ftab, wt=wt, gidx=gidx_all[c], sidx=sidx_all[c], gb=gb)
        for c in range(n_cores)
    ]
    return nc, in_maps, plan


def kernel(feats, W, gamma, beta, in_map, out_map, n_out):
    from concourse.bass_utils import run_bass_kernel_spmd

    n_cores = 8
    nc, in_maps, plan = _prepare(
        feats, W, gamma, beta, in_map, out_map, n_out, n_cores)
    res = run_bass_kernel_spmd(nc, in_maps, list(range(n_cores)))
    rows = plan["rows_per_core"]
    out = np.concatenate(
        [np.asarray(res.results[c]["y"]).reshape(-1, 64)[:rows]
         for c in range(n_cores)], axis=0)
    return out.astype(np.float32)
